# revision 1
# baseline (speedup 1.0000x reference)
"""Trainium2 Bass kernel for nn_GATNodeScorer (GNN message passing).

Strategy (8 NeuronCores, node-partitioned):
  - Host: permute nodes into 160 balanced (core, tile) bins of 128 slots so
    every tile has <= K*128 in-edges; pack edges into 128-edge chunks per
    destination tile; fold attention projections and biases into augmented
    weight matrices.
  - Device, per core (SPMD, one NEFF):
      1. input projection  h = relu(xc @ Wp + bp)    (slab of 2560 nodes)
      2. AllGather H table (f32) across 8 cores
      3. relational layer  h1 = h + segsum(h[src] + rel_emb[type]*w)
         via per-chunk [P,1] indirect-DMA gathers + one-hot f32r matmul
         scatter-add
      4. dense x1 = h1 @ [W1 | W1@Asrc | W1@Adst] in f32r  -> XA table
         (bf16 x, attention logits as bf16 hi/lo pairs), AllGather
      5. GAT layer: per-chunk gathers of [x | as_hi | as_lo] rows by edge
         src; per-edge a_dst via transposed one-hot matmuls (exact f32);
         segment softmax via exp with the denominator columns merged into
         the numerator matmul rhs [msg(256) | ex(4)]; exp() is expanded
         across the 64 head channels on the ACT engine so the bf16 message
         scaling runs in DVE 2x mode
      6. repeat 4-5 for layer 2, then score = h3 @ Wo + bo
  - bf16 message data plane, f32r dense path, f32 PSUM accumulation.

Self-contained: hardcodes all shapes; only needs numpy + the concourse repo
installed at /opt/trn_rl_repo.
"""

import sys

sys.path.insert(0, "/opt/trn_rl_repo")

import heapq

import numpy as np
import ml_dtypes

import concourse.bass as bass
import concourse.bacc as bacc
import concourse.mybir as mybir
import concourse.tile as tile
from concourse.bass_utils import run_bass_kernel_spmd
from concourse.masks import make_identity

# ---- problem constants (hardcoded per contest rules) ----
N, E = 20000, 320000
IN_DIM, CODE_DIM, HIDDEN, HEADS, NREL = 896, 768, 256, 4, 5
CH = HIDDEN // HEADS
CODE_WEIGHT = 3.0
NEG_SLOPE = 0.2

NCORES = 8
P = 128
T = 20  # tiles per core
NTILES = NCORES * T  # 160
NP = T * P  # 2560 padded nodes per core
NPAD = NTILES * P  # 20480
KPROJ = IN_DIM // P  # 7

F32 = mybir.dt.float32
F32R = mybir.dt.float32r
BF16 = mybir.dt.bfloat16
I32 = mybir.dt.int32
NPBF = np.dtype(ml_dtypes.bfloat16)

XAW = HIDDEN + 2 * HEADS  # 264 dense output: [x | a_src | a_dst]
TBLW = HIDDEN + 2 * HEADS  # 264 shared table row: [x | as_hi | as_lo]
MW = HIDDEN + HEADS  # 260 merged matmul rhs: [msg | ex]
NRELP = 6  # NREL padded even

# ---------------------------------------------------------------------------
# host-side planning
# ---------------------------------------------------------------------------


def _pack_nodes(deg_gat, deg_rel, cap_g, cap_r):
    order = np.argsort(-deg_gat, kind="stable")
    load_g = np.zeros(NTILES, np.int64)
    load_r = np.zeros(NTILES, np.int64)
    count = np.zeros(NTILES, np.int64)
    tile_of = np.full(N, -1, np.int64)
    heap = [(0, t) for t in range(NTILES)]
    heapq.heapify(heap)
    for n in order:
        dg, dr = deg_gat[n], deg_rel[n]
        popped = []
        placed = False
        while heap:
            lg, t = heapq.heappop(heap)
            if lg != load_g[t]:
                continue
            if count[t] < P and load_g[t] + dg <= cap_g and load_r[t] + dr <= cap_r:
                tile_of[n] = t
                load_g[t] += dg
                load_r[t] += dr
                count[t] += 1
                if count[t] < P:
                    heapq.heappush(heap, (load_g[t], t))
                placed = True
                break
            popped.append((lg, t))
        for item in popped:
            heapq.heappush(heap, item)
        if not placed:
            raise RuntimeError("packing failed")
    return tile_of


def _pack_edges(src_pp, dst_pp, K):
    tile_e = dst_pp // P
    order_e = np.argsort(tile_e, kind="stable")
    esrc = np.zeros((NTILES, K * P), np.int32)
    dloc = np.full((NTILES, K * P), P, np.float32)
    eord = np.full((NTILES, K * P), -1, np.int64)
    bounds = np.searchsorted(tile_e[order_e], np.arange(NTILES + 1))
    for t in range(NTILES):
        lo, hi = bounds[t], bounds[t + 1]
        ecnt = hi - lo
        if ecnt > K * P:
            raise RuntimeError(f"tile {t}: {ecnt} edges > {K * P}")
        idxs = order_e[lo:hi]
        esrc[t, :ecnt] = src_pp[idxs]
        dloc[t, :ecnt] = (dst_pp[idxs] - t * P).astype(np.float32)
        eord[t, :ecnt] = idxs
    esrc = np.ascontiguousarray(esrc.reshape(NTILES, K, P).transpose(0, 2, 1))
    dloc = np.ascontiguousarray(dloc.reshape(NTILES, K, P).transpose(0, 2, 1))
    eord = np.ascontiguousarray(eord.reshape(NTILES, K, P).transpose(0, 2, 1))
    return esrc, dloc, eord


def _build_plan(edge_index):
    src = edge_index[0].astype(np.int64)
    dst = edge_index[1].astype(np.int64)
    deg_rel = np.bincount(dst, minlength=N)
    # GAT self-loops are handled analytically on-device (x, a_src, a_dst of
    # a node are all core-local), so the gathered edge set equals the rel
    # edge set and both stages share one table.
    for KK in (16, 17, 18):
        try:
            tile_of = _pack_nodes(deg_rel, deg_rel, KK * P, KK * P)
            break
        except RuntimeError:
            continue
    else:
        raise RuntimeError("node packing failed at all K")

    perm = np.full(N, -1, np.int64)
    slot_ctr = np.zeros(NTILES, np.int64)
    for n in np.argsort(tile_of, kind="stable"):
        t = tile_of[n]
        perm[n] = t * P + slot_ctr[t]
        slot_ctr[t] += 1

    src_p, dst_p = perm[src], perm[dst]
    esrc_r, dloc_r, eord_r = _pack_edges(src_p, dst_p, KK)
    return dict(
        perm=perm,
        K_G=KK,
        K_R=KK,
        esrc_r=esrc_r,
        dloc_r=dloc_r,
        eord_r=eord_r,
        esrc_g=esrc_r,
        dloc_g=dloc_r,
    )


def _asrc_mat(att):
    """[HEADS, CH] -> [HIDDEN, HEADS] block matrix so x @ A == (x*att).sum(-1)."""
    A = np.zeros((HIDDEN, HEADS), np.float32)
    for h in range(HEADS):
        A[h * CH : (h + 1) * CH, h] = att[h]
    return A


# ---------------------------------------------------------------------------
# bass program
# ---------------------------------------------------------------------------


def _build_bass(K_R, K_G, probe=None):
    probe = probe or {}
    reps = probe.get("reps", 1)
    nc = bacc.Bacc("TRN2", target_bir_lowering=False, debug=False, num_devices=NCORES)

    # ---- external inputs ----
    xtt_in = nc.dram_tensor("xtt", [T, KPROJ, P, P], BF16, kind="ExternalInput")
    wp_in = nc.dram_tensor("wp", [KPROJ, P, HIDDEN], BF16, kind="ExternalInput")
    bp_in = nc.dram_tensor("bp_row", [1, HIDDEN], F32, kind="ExternalInput")
    w1_in = nc.dram_tensor("w1aug", [2, P, XAW], F32, kind="ExternalInput")
    w2_in = nc.dram_tensor("w2aug", [2, P, XAW], F32, kind="ExternalInput")
    b1w2_in = nc.dram_tensor("b1w2_row", [1, XAW], F32, kind="ExternalInput")
    rel_in = nc.dram_tensor("rel_emb", [NRELP, HIDDEN], F32, kind="ExternalInput")
    worep_in = nc.dram_tensor("wo_rep", [P, HIDDEN], F32, kind="ExternalInput")
    scb_in = nc.dram_tensor("sc_bias", [P, 1], F32, kind="ExternalInput")
    esrc_r_in = nc.dram_tensor("esrc_r", [T, P, K_R], I32, kind="ExternalInput")
    dloc_r_in = nc.dram_tensor("dloc_r", [T, P, K_R], BF16, kind="ExternalInput")
    wtyp_r_in = nc.dram_tensor("wtyp_r", [T, P, NRELP * K_R], BF16, kind="ExternalInput")
    esrc_g_in = nc.dram_tensor("esrc_g", [T, P, K_G], I32, kind="ExternalInput")
    dloc_g_in = nc.dram_tensor("dloc_g", [T, P, K_G], F32, kind="ExternalInput")

    score_out = nc.dram_tensor("score", [NP], F32, kind="ExternalOutput")

    with tile.TileContext(nc) as tc:
        with (
            tc.tile_pool(name="const", bufs=1) as cpool,
            tc.tile_pool(name="hres", bufs=1) as hpool,
            tc.tile_pool(name="lhsT", bufs=4) as lpool,
            tc.tile_pool(name="edge_idx", bufs=3) as epool,
            tc.tile_pool(name="gather", bufs=3) as gpool,
            tc.tile_pool(name="onehot", bufs=2) as opool,
            tc.tile_pool(name="msg", bufs=2) as mpool,
            tc.tile_pool(name="small", bufs=4) as spool,
            tc.tile_pool(name="ps", bufs=1, space="PSUM") as pspool,
            tc.tile_pool(name="dram", bufs=1, space="DRAM") as dpool,
        ):
            # ---- constants ----
            ident = cpool.tile([P, P], F32)
            make_identity(nc, ident[:])
            ident_bf = cpool.tile([P, P], BF16)
            nc.vector.tensor_copy(ident_bf[:], ident[:])
            iota_row_i = cpool.tile([P, P], I32)
            nc.gpsimd.iota(iota_row_i[:], pattern=[[1, P]], base=0, channel_multiplier=0)
            iota_row = cpool.tile([P, P], BF16)
            nc.vector.tensor_copy(iota_row[:], iota_row_i[:])
            iota_row_f = cpool.tile([P, P], F32)
            nc.vector.tensor_copy(iota_row_f[:], iota_row_i[:])
            iota_col_i = cpool.tile([P, 1], I32)
            nc.gpsimd.iota(iota_col_i[:], pattern=[[0, 1]], base=0, channel_multiplier=1)
            iota_col = cpool.tile([P, 1], F32)
            nc.vector.tensor_copy(iota_col[:], iota_col_i[:])
            ones_row = cpool.tile([1, P], F32)
            nc.vector.memset(ones_row[:], 1.0)

            # weights resident in SBUF (proj bf16; GAT dense f32r since exp
            # amplifies its rounding)
            w_scr = cpool.tile([P, 2 * XAW], F32)
            wp_sb = cpool.tile([P, KPROJ * HIDDEN], BF16)
            for k in range(KPROJ):
                nc.sync.dma_start(
                    wp_sb[:, k * HIDDEN : (k + 1) * HIDDEN], wp_in[k, :, :]
                )
            bp_sb = cpool.tile([1, HIDDEN], F32)
            nc.sync.dma_start(bp_sb[:], bp_in[:, :])

            waug = []
            for li, w_in in enumerate((w1_in, w2_in)):
                wr = cpool.tile([P, 2 * XAW], F32R, name=f"w{li}")
                for k in range(2):
                    nc.sync.dma_start(w_scr[:, k * XAW : (k + 1) * XAW], w_in[k, :, :])
                nc.vector.tensor_copy(wr[:], w_scr[:])
                waug.append(wr)

            b1w2_sb = cpool.tile([1, XAW], F32)
            nc.sync.dma_start(b1w2_sb[:], b1w2_in[:, :])
            rel_f = cpool.tile([NRELP, HIDDEN], F32)
            nc.sync.dma_start(rel_f[:], rel_in[:, :])
            rel_sb = cpool.tile([NRELP, HIDDEN], BF16)
            nc.vector.tensor_copy(rel_sb[:], rel_f[:])
            worep_sb = cpool.tile([P, HIDDEN], F32)
            nc.sync.dma_start(worep_sb[:], worep_in[:, :])
            scb_sb = cpool.tile([P, 1], F32)
            nc.sync.dma_start(scb_sb[:], scb_in[:, :])

            # residual h slabs (two ping-pong slabs of T tiles, f32)
            hA = hpool.tile([P, T * HIDDEN], F32)
            hB = hpool.tile([P, T * HIDDEN], F32)
            # resident per-tile a_src/a_dst columns (f32, exact) and x rows
            # (bf16) for the analytic self-loop contribution
            adst_all = hpool.tile([P, T * HEADS], F32)
            asrc_all = hpool.tile([P, T * HEADS], F32)
            x_all = hpool.tile([P, T * HIDDEN], BF16)

            # DRAM bounce buffers for collectives (bf16).  A Shared tensor
            # may only be written by one instruction, so timing builds
            # (reps > 1) get per-rep tables.
            h_slab = dpool.tile([NP, HIDDEN], BF16)
            xa_slab = dpool.tile([NP, TBLW], BF16)
            xa_slab2 = dpool.tile([NP, TBLW], BF16)
            h_fulls = [
                dpool.tile([NPAD, HIDDEN], BF16, addr_space="Shared", name=f"h_full{r}")
                for r in range(reps)
            ]
            xa_fulls = [
                dpool.tile([NPAD, TBLW], BF16, addr_space="Shared", name=f"xa_full{r}")
                for r in range(reps)
            ]
            xa_full2s = [
                dpool.tile([NPAD, TBLW], BF16, addr_space="Shared", name=f"xa_full2{r}")
                for r in range(reps)
            ]

            def hcols(t):
                return slice(t * HIDDEN, (t + 1) * HIDDEN)

            for rep in range(reps):
                h_full = h_fulls[rep]
                xa_full = xa_fulls[rep]
                xa_full2 = xa_full2s[rep]
                # ================= stage 1: input projection =================
                for t in range(T):
                    proj_ps = pspool.tile([P, HIDDEN], F32, tag="work", bufs=2)
                    for k in range(KPROJ):
                        lx = lpool.tile([P, P], BF16, tag="lhsT")
                        nc.sync.dma_start(lx[:], xtt_in[t, k, :, :])
                        nc.tensor.matmul(
                            out=proj_ps[:],
                            lhsT=lx[:],
                            rhs=wp_sb[:, k * HIDDEN : (k + 1) * HIDDEN],
                            start=(k == 0),
                            stop=False,
                        )
                    nc.tensor.matmul(
                        out=proj_ps[:],
                        lhsT=ones_row[:1, :],
                        rhs=bp_sb[:1, :],
                        start=False,
                        stop=True,
                    )
                    nc.scalar.activation(
                        out=hA[:, hcols(t)],
                        in_=proj_ps[:],
                        func=mybir.ActivationFunctionType.Relu,
                    )
                    hsl = spool.tile([P, HIDDEN], BF16, tag="hsl")
                    nc.vector.tensor_copy(hsl[:], hA[:, hcols(t)])
                    nc.sync.dma_start(h_slab[t * P : (t + 1) * P, :], hsl[:])

                if probe.get("stop_after") == "proj":
                    continue
                # ================= AllGather H =================
                if probe.get("no_collective"):
                    nc.sync.dma_start(h_full[0:NP, :], h_slab[:, :])
                else:
                    nc.gpsimd.collective_compute(
                        "AllGather",
                        mybir.AluOpType.bypass,
                        replica_groups=[list(range(NCORES))],
                        ins=[h_slab.opt()],
                        outs=[h_full.opt()],
                    )

                # ================= stage 2: relational layer =================
                for t in range(T):
                    esrc_t = epool.tile([P, K_R], I32, tag="esrc")
                    nc.sync.dma_start(esrc_t[:], esrc_r_in[t, :, :])
                    dloc_t = epool.tile([P, K_R], BF16, tag="dlocb")
                    nc.sync.dma_start(dloc_t[:], dloc_r_in[t, :, :])
                    wt_t = epool.tile([P, NRELP * K_R], BF16, tag="wtyp")
                    nc.sync.dma_start(wt_t[:], wtyp_r_in[t, :, :])

                    # batched gather of all K_R chunks for this tile (bf16)
                    hch = gpool.tile([P, K_R * HIDDEN], BF16, tag="gather")
                    for k in range(K_R):
                        nc.gpsimd.indirect_dma_start(
                            out=hch[:, k * HIDDEN : (k + 1) * HIDDEN],
                            out_offset=None,
                            in_=h_full[:, :],
                            in_offset=bass.IndirectOffsetOnAxis(
                                ap=esrc_t[:, k : k + 1], axis=0
                            ),
                        )
                    # all one-hots in one DVE op
                    oh = opool.tile([P, K_R * P], BF16, tag="onehot")
                    nc.vector.tensor_tensor(
                        out=oh[:].rearrange("p (k e) -> p k e", k=K_R),
                        in0=dloc_t[:].unsqueeze(-1).to_broadcast([P, K_R, P]),
                        in1=iota_row[:].unsqueeze(1).to_broadcast([P, K_R, P]),
                        op=mybir.AluOpType.is_equal,
                    )
                    out_ps = pspool.tile([P, HIDDEN], F32, tag="out", bufs=2)
                    wm_ps = pspool.tile([P, NRELP], F32, tag="acc4", bufs=1)
                    for k in range(K_R):
                        nc.tensor.matmul(
                            out=out_ps[:],
                            lhsT=oh[:, k * P : (k + 1) * P],
                            rhs=hch[:, k * HIDDEN : (k + 1) * HIDDEN],
                            start=(k == 0),
                            stop=(k == K_R - 1),
                        )
                        nc.tensor.matmul(
                            out=wm_ps[:],
                            lhsT=oh[:, k * P : (k + 1) * P],
                            rhs=wt_t[:, k * NRELP : (k + 1) * NRELP],
                            start=(k == 0),
                            stop=(k == K_R - 1),
                        )
                    # rel contribution: wmatT [6, P] then rel_embT matmul
                    wmat_sb = spool.tile([P, NRELP], BF16, tag="wmat")
                    nc.vector.tensor_copy(wmat_sb[:], wm_ps[:])
                    wmatT_ps = pspool.tile([NRELP, P], BF16, tag="tmp", bufs=2)
                    nc.tensor.transpose(
                        out=wmatT_ps[:], in_=wmat_sb[:], identity=ident_bf[:]
                    )
                    wmatT_sb = spool.tile([NRELP, P], BF16, tag="wmatT")
                    nc.vector.tensor_copy(wmatT_sb[:], wmatT_ps[:])
                    rel_ps = pspool.tile([P, HIDDEN], F32, tag="work", bufs=2)
                    nc.tensor.matmul(
                        out=rel_ps[:],
                        lhsT=wmatT_sb[:],
                        rhs=rel_sb[:],
                        start=True,
                        stop=True,
                    )
                    # h1 = h + segsum + rel  (one PSUM operand per DVE op)
                    tsum = spool.tile([P, HIDDEN], F32, tag="tsum")
                    nc.vector.tensor_add(tsum[:], out_ps[:], hA[:, hcols(t)])
                    nc.vector.tensor_add(hB[:, hcols(t)], rel_ps[:], tsum[:])

                if probe.get("stop_after") == "rel":
                    continue
                # ============ stages 3/4: GAT layers ============
                for layer in range(2):
                    hin = hB if layer == 0 else hA
                    hout = hA if layer == 0 else hB
                    wr = waug[layer]
                    slab = xa_slab if layer == 0 else xa_slab2
                    full = xa_full if layer == 0 else xa_full2

                    # ---- dense: x = h @ Waug (+ b-fold for layer 1) ----
                    for t in range(T):
                        x_ps = pspool.tile([P, XAW], F32, tag="work", bufs=2)
                        for half in range(2):
                            tr_ps = pspool.tile([P, P], F32, tag="tmp", bufs=2)
                            nc.tensor.transpose(
                                out=tr_ps[:],
                                in_=hin[
                                    :,
                                    t * HIDDEN + half * P : t * HIDDEN + (half + 1) * P,
                                ],
                                identity=ident[:],
                            )
                            ht_r = lpool.tile([P, P], F32R, tag="lhsTr")
                            nc.vector.tensor_copy(ht_r[:], tr_ps[:])
                            nc.tensor.matmul(
                                out=x_ps[:],
                                lhsT=ht_r[:],
                                rhs=wr[:, half * XAW : (half + 1) * XAW],
                                start=(half == 0),
                                stop=(half == 1 and layer == 0),
                            )
                        if layer == 1:
                            # fold h2 = gat1_out + b1 into x2 = h2 @ W2aug
                            nc.tensor.matmul(
                                out=x_ps[:],
                                lhsT=ones_row[:1, :],
                                rhs=b1w2_sb[:1, :],
                                start=False,
                                stop=True,
                            )
                        # shared-table row [x(256) | as_hi | as_lo]; a_src is
                        # stored as a bf16 hi/lo split of the f32 logits
                        # (exp() amplifies rounding).  a_dst stays resident
                        # in f32 (only needed for local dst nodes).
                        xa_sb = gpool.tile([P, TBLW], BF16, tag="xa_sb")
                        nc.vector.tensor_copy(xa_sb[:, 0:HIDDEN], x_ps[:, 0:HIDDEN])
                        as_ps = x_ps[:, HIDDEN : HIDDEN + HEADS]
                        hi_ap = xa_sb[:, HIDDEN : HIDDEN + HEADS]
                        lo_ap = xa_sb[:, HIDDEN + HEADS : HIDDEN + 2 * HEADS]
                        nc.vector.tensor_copy(hi_ap, as_ps)
                        hi32 = spool.tile([P, HEADS], F32, tag="hi32")
                        nc.vector.tensor_copy(hi32[:], hi_ap)
                        nc.vector.tensor_tensor(
                            out=lo_ap, in0=as_ps, in1=hi32[:],
                            op=mybir.AluOpType.subtract,
                        )
                        nc.vector.tensor_copy(
                            adst_all[:, t * HEADS : (t + 1) * HEADS],
                            x_ps[:, HIDDEN + HEADS : XAW],
                        )
                        nc.vector.tensor_copy(
                            asrc_all[:, t * HEADS : (t + 1) * HEADS],
                            x_ps[:, HIDDEN : HIDDEN + HEADS],
                        )
                        nc.vector.tensor_copy(
                            x_all[:, hcols(t)], xa_sb[:, 0:HIDDEN]
                        )
                        nc.sync.dma_start(slab[t * P : (t + 1) * P, :], xa_sb[:])

                    if probe.get("no_collective"):
                        nc.sync.dma_start(full[0:NP, :], slab[:, :])
                    else:
                        nc.gpsimd.collective_compute(
                            "AllGather",
                            mybir.AluOpType.bypass,
                            replica_groups=[list(range(NCORES))],
                            ins=[slab.opt()],
                            outs=[full.opt()],
                        )

                    # ---- edge stage ----
                    if probe.get("stop_after") == f"dense{layer + 1}":
                        break
                    for t in range(T):
                        esrc_t = epool.tile([P, K_G], I32, tag="esrc")
                        nc.sync.dma_start(esrc_t[:], esrc_g_in[t, :, :])
                        dloc_t = epool.tile([P, K_G], F32, tag="dloc")
                        nc.sync.dma_start(dloc_t[:], dloc_g_in[t, :, :])
                        dloc_bf = epool.tile([P, K_G], BF16, tag="dlocb")
                        nc.vector.tensor_copy(dloc_bf[:], dloc_t[:])

                        # batched gather: [x | as_hi | as_lo] rows by src
                        xa_all = gpool.tile([P, K_G * TBLW], BF16, tag="gather")
                        xa_v = xa_all[:].rearrange("p (k w) -> p k w", k=K_G)
                        for k in range(K_G):
                            nc.gpsimd.indirect_dma_start(
                                out=xa_all[:, k * TBLW : (k + 1) * TBLW],
                                out_offset=None,
                                in_=full[:, :],
                                in_offset=bass.IndirectOffsetOnAxis(
                                    ap=esrc_t[:, k : k + 1], axis=0
                                ),
                            )
                        # per-edge a_dst via transposed one-hots (exact f32)
                        ea_ps = pspool.tile([P, K_G * HEADS], F32, tag="ea", bufs=1)
                        for k in range(K_G):
                            row_ps = pspool.tile([P, P], F32, tag="tmp", bufs=2)
                            nc.tensor.transpose(
                                out=row_ps[:],
                                in_=dloc_t[:, k : k + 1].to_broadcast([P, P]),
                                identity=ident[:],
                            )
                            ohT = opool.tile([P, P], F32, tag="onehotT")
                            nc.vector.tensor_tensor(
                                out=ohT[:],
                                in0=iota_col[:].to_broadcast([P, P]),
                                in1=row_ps[:],
                                op=mybir.AluOpType.is_equal,
                            )
                            nc.tensor.matmul(
                                out=ea_ps[:, k * HEADS : (k + 1) * HEADS],
                                lhsT=ohT[:],
                                rhs=adst_all[:, t * HEADS : (t + 1) * HEADS],
                                start=True,
                                stop=True,
                            )
                        # alpha = (as_hi + as_lo) + ea   [P, K, 4] f32
                        a1 = spool.tile([P, K_G * HEADS], F32, tag="a1")
                        nc.vector.tensor_tensor(
                            out=a1[:].rearrange("p (k h) -> p k h", k=K_G),
                            in0=xa_v[:, :, HIDDEN : HIDDEN + HEADS],
                            in1=xa_v[:, :, HIDDEN + HEADS : HIDDEN + 2 * HEADS],
                            op=mybir.AluOpType.add,
                        )
                        alpha = spool.tile([P, K_G * HEADS], F32, tag="alpha")
                        nc.vector.tensor_add(alpha[:], a1[:], ea_ps[:])
                        # leaky relu: max(alpha, slope*alpha) on DVE
                        asc = spool.tile([P, K_G * HEADS], F32, tag="asc")
                        nc.vector.tensor_scalar_mul(asc[:], alpha[:], NEG_SLOPE)
                        lr = spool.tile([P, K_G * HEADS], F32, tag="lr")
                        nc.vector.tensor_tensor(
                            out=lr[:], in0=alpha[:], in1=asc[:], op=mybir.AluOpType.max
                        )
                        # merged rhs [msg(256) | ex(4)] per chunk
                        mg = mpool.tile([P, K_G * MW], BF16, tag="msg")
                        mg_v = mg[:].rearrange("p (k w) -> p k w", k=K_G)
                        nc.scalar.activation(
                            out=mg_v[:, :, HIDDEN:MW],
                            in_=lr[:].rearrange("p (k h) -> p k h", k=K_G),
                            func=mybir.ActivationFunctionType.Exp,
                        )
                        # ex expanded across the 64 head channels (ACT)
                        ex_rep = mpool.tile([P, K_G * HIDDEN], BF16, tag="ex_rep")
                        if probe.get("no_exprep"):
                            nc.vector.memset(ex_rep[:], 1.0)
                        else:
                            nc.scalar.activation(
                                out=ex_rep[:].rearrange(
                                    "p (k h c) -> p k h c", k=K_G, h=HEADS
                                ),
                                in_=lr[:]
                                .rearrange("p (k h) -> p k h", k=K_G)
                                .unsqueeze(-1)
                                .to_broadcast([P, K_G, HEADS, CH]),
                                func=mybir.ActivationFunctionType.Exp,
                            )
                        # msg = x * ex  (all-bf16 packed -> DVE 2x mode)
                        nc.vector.tensor_tensor(
                            out=mg_v[:, :, 0:HIDDEN],
                            in0=xa_v[:, :, 0:HIDDEN],
                            in1=ex_rep[:].rearrange("p (k c) -> p k c", k=K_G),
                            op=mybir.AluOpType.mult,
                        )
                        # one-hots
                        oh = opool.tile([P, K_G * P], BF16, tag="onehot")
                        nc.vector.tensor_tensor(
                            out=oh[:].rearrange("p (k e) -> p k e", k=K_G),
                            in0=dloc_bf[:].unsqueeze(-1).to_broadcast([P, K_G, P]),
                            in1=iota_row[:].unsqueeze(1).to_broadcast([P, K_G, P]),
                            op=mybir.AluOpType.is_equal,
                        )
                        # accumulation streak on PE: [num(256) | den(4)]
                        out_ps = pspool.tile([P, MW], F32, tag="out", bufs=2)
                        for k in range(K_G):
                            nc.tensor.matmul(
                                out=out_ps[:],
                                lhsT=oh[:, k * P : (k + 1) * P],
                                rhs=mg[:, k * MW : (k + 1) * MW],
                                start=(k == 0),
                                stop=(k == K_G - 1),
                            )
                        # analytic self-loop contribution (x, a_src, a_dst all
                        # local; no gather, no one-hot, no edge slot)
                        a_s = spool.tile([P, HEADS], F32, tag="a_s")
                        nc.vector.tensor_add(
                            a_s[:],
                            asrc_all[:, t * HEADS : (t + 1) * HEADS],
                            adst_all[:, t * HEADS : (t + 1) * HEADS],
                        )
                        a_sc = spool.tile([P, HEADS], F32, tag="a_sc")
                        nc.vector.tensor_scalar_mul(a_sc[:], a_s[:], NEG_SLOPE)
                        lr_s = spool.tile([P, HEADS], F32, tag="lr_s")
                        nc.vector.tensor_tensor(
                            out=lr_s[:], in0=a_s[:], in1=a_sc[:],
                            op=mybir.AluOpType.max,
                        )
                        ex_s = spool.tile([P, HEADS], F32, tag="ex_s")
                        nc.scalar.activation(
                            out=ex_s[:],
                            in_=lr_s[:],
                            func=mybir.ActivationFunctionType.Exp,
                        )
                        smsg = spool.tile([P, MW], F32, tag="smsg")
                        nc.vector.tensor_tensor(
                            out=smsg[:, 0:HIDDEN].rearrange("p (h c) -> p h c", h=HEADS),
                            in0=x_all[:, hcols(t)].rearrange("p (h c) -> p h c", h=HEADS),
                            in1=ex_s[:].unsqueeze(-1).to_broadcast([P, HEADS, CH]),
                            op=mybir.AluOpType.mult,
                        )
                        nc.vector.tensor_copy(smsg[:, HIDDEN:MW], ex_s[:])
                        tot = spool.tile([P, MW], F32, tag="tot")
                        nc.vector.tensor_add(tot[:], out_ps[:], smsg[:])
                        # normalize: h_next = num / den
                        den = spool.tile([P, HEADS], F32, tag="den")
                        nc.vector.tensor_scalar_add(
                            den[:], tot[:, HIDDEN:MW], 1e-30
                        )
                        dinv = spool.tile([P, HEADS], F32, tag="dinv")
                        nc.vector.reciprocal(dinv[:], den[:])
                        nc.vector.tensor_tensor(
                            out=hout[:, hcols(t)].rearrange("p (h c) -> p h c", h=HEADS),
                            in0=tot[:, 0:HIDDEN].rearrange("p (h c) -> p h c", h=HEADS),
                            in1=dinv[:].unsqueeze(-1).to_broadcast([P, HEADS, CH]),
                            op=mybir.AluOpType.mult,
                        )

                    if probe.get("stop_after") == f"gat{layer + 1}":
                        break
                if probe.get("stop_after") in ("dense1", "gat1", "dense2", "gat2"):
                    continue
                # ================= stage 5: score =================
                for t in range(T):
                    prod = spool.tile([P, HIDDEN], F32, tag="tsum")
                    nc.vector.tensor_mul(prod[:], hB[:, hcols(t)], worep_sb[:])
                    red = spool.tile([P, 1], F32, tag="red")
                    nc.vector.tensor_reduce(
                        out=red[:],
                        in_=prod[:],
                        axis=mybir.AxisListType.X,
                        op=mybir.AluOpType.add,
                    )
                    sc = spool.tile([P, 1], F32, tag="sc")
                    nc.vector.tensor_add(sc[:], red[:], scb_sb[:])
                    nc.sync.dma_start(score_out[t * P : (t + 1) * P], sc[:])

    nc.compile()
    return nc


# ---------------------------------------------------------------------------
# entry point
# ---------------------------------------------------------------------------

_CACHE = {}


def prepare(inputs, plan, probe=None):
    """Build (in_maps, nc, perm) from the full input dict + plan."""
    x = np.asarray(inputs["x"], np.float32)
    edge_type = np.asarray(inputs["edge_type"], np.int32)
    edge_weight = np.asarray(inputs["edge_weight"], np.float32)
    rel_emb = np.asarray(inputs["rel_emb"], np.float32)
    Wp = np.asarray(inputs["Wp"], np.float32)
    bp = np.asarray(inputs["bp"], np.float32)
    W1 = np.asarray(inputs["W1"], np.float32)
    W2 = np.asarray(inputs["W2"], np.float32)
    att_src1 = np.asarray(inputs["att_src1"], np.float32)
    att_dst1 = np.asarray(inputs["att_dst1"], np.float32)
    att_src2 = np.asarray(inputs["att_src2"], np.float32)
    att_dst2 = np.asarray(inputs["att_dst2"], np.float32)
    b1 = np.asarray(inputs["b1"], np.float32)
    b2 = np.asarray(inputs["b2"], np.float32)
    Wo = np.asarray(inputs["Wo"], np.float32)
    bo = np.asarray(inputs["bo"], np.float32)

    perm = plan["perm"]
    K_R, K_G = plan["K_R"], plan["K_G"]

    # ---- per-core dense inputs ----
    xr = np.concatenate([x[:, CODE_DIM:], CODE_WEIGHT * x[:, :CODE_DIM]], axis=1)
    xpad = np.zeros((NPAD, IN_DIM), np.float32)
    xpad[perm] = xr
    # [NCORES, T, KPROJ, P(feat), P(node)]
    xtt = (
        xpad.reshape(NCORES, T, P, KPROJ, P).transpose(0, 1, 3, 4, 2).astype(NPBF)
    )

    w1aug = np.concatenate(
        [W1, W1 @ _asrc_mat(att_src1), W1 @ _asrc_mat(att_dst1)], axis=1
    )
    w2aug = np.concatenate(
        [W2, W2 @ _asrc_mat(att_src2), W2 @ _asrc_mat(att_dst2)], axis=1
    )
    b1w2 = (b1 @ w2aug).reshape(1, XAW).astype(np.float32)
    sc_bias = float(b2 @ Wo[:, 0] + bo[0])

    # ---- per-edge rel wtype rows: w_e * onehot6(type_e) ----
    eord_r = plan["eord_r"]  # [NTILES, P, K_R]
    wtyp = np.zeros((NTILES, P, K_R, NRELP), np.float32)
    valid = eord_r >= 0
    ew = np.where(valid, edge_weight[np.clip(eord_r, 0, E - 1)], 0.0).astype(np.float32)
    et = np.where(valid, edge_type[np.clip(eord_r, 0, E - 1)], 0)
    ii, jj, kk = np.nonzero(valid)
    wtyp[ii, jj, kk, et[ii, jj, kk]] = ew[ii, jj, kk]
    wtyp = wtyp.reshape(NTILES, P, K_R * NRELP)

    key = (K_R, K_G, tuple(sorted((probe or {}).items())))
    if key not in _CACHE:
        _CACHE[key] = _build_bass(K_R, K_G, probe)
    nc = _CACHE[key]

    common = dict(
        wp=np.ascontiguousarray(Wp.reshape(KPROJ, P, HIDDEN)).astype(NPBF),
        bp_row=bp.reshape(1, HIDDEN),
        w1aug=np.ascontiguousarray(w1aug.reshape(2, P, XAW)),
        w2aug=np.ascontiguousarray(w2aug.reshape(2, P, XAW)),
        b1w2_row=b1w2,
        rel_emb=np.concatenate(
            [rel_emb, np.zeros((NRELP - NREL, HIDDEN), np.float32)]
        ),
        wo_rep=np.ascontiguousarray(np.broadcast_to(Wo[:, 0], (P, HIDDEN))),
        sc_bias=np.full((P, 1), sc_bias, np.float32),
    )
    in_maps = []
    for c in range(NCORES):
        ts = slice(c * T, (c + 1) * T)
        in_maps.append(
            dict(
                common,
                xtt=xtt[c],
                esrc_r=plan["esrc_r"][ts],
                dloc_r=plan["dloc_r"][ts].astype(NPBF),
                wtyp_r=np.ascontiguousarray(wtyp[ts]).astype(NPBF),
                esrc_g=plan["esrc_g"][ts],
                dloc_g=plan["dloc_g"][ts],
            )
        )
    return in_maps, nc, perm


def kernel(x, edge_index, **rest):
    inputs = dict(rest, x=x, edge_index=edge_index)
    edge_index = np.asarray(edge_index, np.int32)
    plan = _build_plan(edge_index)
    in_maps, nc, perm = prepare(inputs, plan)

    import os

    trace = bool(os.environ.get("GAT_TRACE"))
    res = run_bass_kernel_spmd(
        nc, in_maps, core_ids=list(range(NCORES)), trace=trace
    )
    global _LAST_RESULT
    _LAST_RESULT = res
    scores_pad = np.concatenate([r["score"] for r in res.results])
    return scores_pad[perm].astype(np.float32)


_LAST_RESULT = None



# revision 8
# speedup vs baseline: 1.0362x; 1.0362x over previous
"""Trainium2 Bass kernel for nn_GATNodeScorer (GNN message passing).

Strategy (8 NeuronCores, node-partitioned, slot-aligned edge packing):
  - Host: sort nodes by in-degree; tile (core, round) gets 125 consecutive
    sorted nodes (+3 spare slots).  All 8 cores' tiles in round j share one
    chunk count K_j = max degree in the round, so the SPMD program is
    identical across cores and per-core work is balanced.  Edges are packed
    so that slot p of chunk k holds an edge whose DESTINATION is slot p:
    segment-sum collapses to a plain reduction over chunks and per-edge
    a_dst is a direct slot lookup -- no one-hot matmuls, no transposes.
  - Device, per core (SPMD, one NEFF):
      1. input projection  h = relu(xc @ Wp + bp)
      2. AllGather H table (bf16), one dma_gather per tile fetches all
         K*128 in-edge rows (512B each) in a single SWDGE instruction
      3. relational layer  h1 = h + tree_sum_k(h[src]) + RW @ rel_emb
         (RW = per-node type/weight histogram, precomputed on host)
      4. dense x1 = h1 @ [W1 | W1@Asrc | W1@Adst]; shared-table row is
         [x bf16(256) | a_src f32 bitcast(8)| pad] = 768B; AllGather
      5. GAT layer: one dma_gather per tile; alpha = a_src(f32) + a_dst;
         ex = exp(leakyrelu(alpha)); msg = x * ex broadcast; merged
         [msg|ex] tree-reduced over chunks; normalize.  Self-loops are
         materialized as chunk 0.  Padding gathers row 127, forced to
         x=0 / a_src=-100 so ex ~ 0.
      6. repeat 4-5 for layer 2, then score = h3 @ Wo + bo

Self-contained: hardcodes all shapes; only needs numpy + the concourse repo
installed at /opt/trn_rl_repo.
"""

import sys

sys.path.insert(0, "/opt/trn_rl_repo")

import numpy as np
import ml_dtypes

import concourse.bass as bass
import concourse.bacc as bacc
import concourse.mybir as mybir
import concourse.tile as tile
from concourse.bass_utils import run_bass_kernel_spmd
from concourse.masks import make_identity

# ---- problem constants (hardcoded per contest rules) ----
N, E = 20000, 320000
IN_DIM, CODE_DIM, HIDDEN, HEADS, NREL = 896, 768, 256, 4, 5
CH = HIDDEN // HEADS
CODE_WEIGHT = 3.0
NEG_SLOPE = 0.2

NCORES = 8
P = 128
T = 20  # rounds (tiles per core)
NTILES = NCORES * T  # 160
NP = T * P  # 2560 padded nodes per core
NPAD = NTILES * P  # 20480
NPT = 125  # real nodes per tile (160*125 = 20000)
MROW = 127  # global row used for padding gathers (forced content)
KPROJ = IN_DIM // P  # 7

F32 = mybir.dt.float32
F32R = mybir.dt.float32r
BF16 = mybir.dt.bfloat16
I16 = mybir.dt.int16
NPBF = np.dtype(ml_dtypes.bfloat16)

XAW = HIDDEN + 2 * HEADS  # 264 dense output: [x | a_src | a_dst]
GW = 384  # gathered GAT table row: [x(256) | a_src f32 as 8 | pad] = 768B
MW = HIDDEN + HEADS  # 260 merged reduce row: [msg | ex]
NRELP = 6

# ---------------------------------------------------------------------------
# host-side planning
# ---------------------------------------------------------------------------


def _build_plan(edge_index):
    src = edge_index[0].astype(np.int64)
    dst = edge_index[1].astype(np.int64)
    indeg = np.bincount(dst, minlength=N)
    order = np.argsort(-indeg, kind="stable")
    ranks = np.empty(N, np.int64)
    ranks[order] = np.arange(N)
    grp = ranks // NPT
    perm = (grp % NCORES) * NP + (grp // NCORES) * P + (ranks % NPT)

    K_rel = np.array(
        [
            int(indeg[order[j * NCORES * NPT : (j + 1) * NCORES * NPT]].max())
            for j in range(T)
        ],
        np.int64,
    )
    K_gat = K_rel + 1

    pd = perm[dst]
    order_e = np.argsort(pd, kind="stable")
    sd = pd[order_e]
    ps = perm[src][order_e].astype(np.int16)
    starts = np.r_[0, np.flatnonzero(np.diff(sd)) + 1]
    kk = np.arange(E, dtype=np.int64) - np.repeat(
        starts, np.diff(np.r_[starts, E])
    )
    ec = sd // NP
    erem = sd % NP
    ej = erem // P
    es = erem % P

    offs_rel = np.r_[0, np.cumsum(K_rel)]
    offs_gat = np.r_[0, np.cumsum(K_gat)]
    SR = 8 * int(K_rel.sum())
    SG = 8 * int(K_gat.sum())
    eidx_rel = np.full((NCORES, 128, SR), MROW, np.int16)
    eidx_gat = np.full((NCORES, 128, SG), MROW, np.int16)

    for j in range(T):
        KG = int(K_gat[j])
        m = ej == j
        A = np.full((NCORES, P, KG), MROW, np.int16)
        sidx = np.arange(NPT)
        for c in range(NCORES):
            A[c, :NPT, 0] = (c * NP + j * P + sidx).astype(np.int16)
        A[ec[m], es[m], kk[m] + 1] = ps[m]
        for c in range(NCORES):
            vg = np.ascontiguousarray(A[c].T).ravel()
            img = np.ascontiguousarray(vg.reshape(-1, 16).T)
            eidx_gat[c, :, 8 * offs_gat[j] : 8 * offs_gat[j + 1]] = np.tile(
                img, (8, 1)
            )
            vr = np.ascontiguousarray(A[c, :, 1:].T).ravel()
            imgr = np.ascontiguousarray(vr.reshape(-1, 16).T)
            eidx_rel[c, :, 8 * offs_rel[j] : 8 * offs_rel[j + 1]] = np.tile(
                imgr, (8, 1)
            )

    return dict(
        perm=perm,
        K_rel=tuple(int(k) for k in K_rel),
        K_gat=tuple(int(k) for k in K_gat),
        offs_rel=tuple(int(o) for o in offs_rel),
        offs_gat=tuple(int(o) for o in offs_gat),
        eidx_rel=eidx_rel,
        eidx_gat=eidx_gat,
    )


def _make_mrow():
    """Padding-target row: x = 0, a_src (f32 bitcast at bf16 cols 256..264)
    = -100 so exp(leakyrelu(alpha)) ~ 0 for padding edges."""
    row = np.zeros((1, GW), NPBF)
    row.view(np.uint8)[0, 2 * HIDDEN : 2 * HIDDEN + 16] = (
        np.full(HEADS, -100.0, np.float32).view(np.uint8)
    )
    return row


def _asrc_mat(att):
    """[HEADS, CH] -> [HIDDEN, HEADS] block matrix so x @ A == (x*att).sum(-1)."""
    A = np.zeros((HIDDEN, HEADS), np.float32)
    for h in range(HEADS):
        A[h * CH : (h + 1) * CH, h] = att[h]
    return A


# ---------------------------------------------------------------------------
# bass program
# ---------------------------------------------------------------------------


def _tree_reduce(nc, src, acc, K, CW):
    """Sum K chunks of width CW from src (bf16 [P, K*CW]) into acc
    (f32 [P, ceil(K/2)*CW]); returns AP [P, CW] f32."""
    h = K // 2
    odd = K % 2
    if h == 0:
        nc.vector.tensor_copy(acc[:, :CW], src[:, :CW])
        return acc[:, :CW]
    nc.vector.tensor_add(acc[:, : h * CW], src[:, : h * CW], src[:, h * CW : 2 * h * CW])
    if odd:
        nc.vector.tensor_copy(
            acc[:, h * CW : (h + 1) * CW], src[:, 2 * h * CW : (2 * h + 1) * CW]
        )
        h += 1
    while h > 1:
        hh = h // 2
        odd = h % 2
        nc.vector.tensor_add(
            acc[:, : hh * CW], acc[:, : hh * CW], acc[:, hh * CW : 2 * hh * CW]
        )
        if odd:
            nc.vector.tensor_add(
                acc[:, :CW], acc[:, :CW], acc[:, 2 * hh * CW : (2 * hh + 1) * CW]
            )
        h = hh
    return acc[:, :CW]


def _build_bass(K_rel, K_gat, offs_rel, offs_gat, probe=None):
    probe = probe or {}
    reps = probe.get("reps", 1)
    Kmax = max(K_gat)
    SR = 8 * sum(K_rel)
    SG = 8 * sum(K_gat)
    nc = bacc.Bacc("TRN2", target_bir_lowering=False, debug=False, num_devices=NCORES)

    # ---- external inputs ----
    xtt_in = nc.dram_tensor("xtt", [T, KPROJ, P, P], BF16, kind="ExternalInput")
    wp_in = nc.dram_tensor("wp", [KPROJ, P, HIDDEN], BF16, kind="ExternalInput")
    bp_in = nc.dram_tensor("bp_row", [1, HIDDEN], F32, kind="ExternalInput")
    w1_in = nc.dram_tensor("w1aug", [2, P, XAW], F32, kind="ExternalInput")
    w2_in = nc.dram_tensor("w2aug", [2, P, XAW], F32, kind="ExternalInput")
    b1w2_in = nc.dram_tensor("b1w2_row", [1, XAW], F32, kind="ExternalInput")
    rel_in = nc.dram_tensor("rel_emb", [NRELP, HIDDEN], F32, kind="ExternalInput")
    rwt_in = nc.dram_tensor("rwT", [NRELP, NP], F32, kind="ExternalInput")
    worep_in = nc.dram_tensor("wo_rep", [P, HIDDEN], F32, kind="ExternalInput")
    scb_in = nc.dram_tensor("sc_bias", [P, 1], F32, kind="ExternalInput")
    er_in = nc.dram_tensor("eidx_rel", [128, SR], I16, kind="ExternalInput")
    eg_in = nc.dram_tensor("eidx_gat", [128, SG], I16, kind="ExternalInput")
    mrow_in = nc.dram_tensor("mrow", [1, GW], BF16, kind="ExternalInput")

    score_out = nc.dram_tensor("score", [NP], F32, kind="ExternalOutput")

    with tile.TileContext(nc) as tc:
        with (
            tc.tile_pool(name="const", bufs=1) as cpool,
            tc.tile_pool(name="hres", bufs=1) as hpool,
            tc.tile_pool(name="lhsT", bufs=4) as lpool,
            tc.tile_pool(name="gather", bufs=2) as gpool,
            tc.tile_pool(name="msg", bufs=2) as mpool,
            tc.tile_pool(name="acc", bufs=1) as apool,
            tc.tile_pool(name="small", bufs=2) as spool,
            tc.tile_pool(name="ps", bufs=1, space="PSUM") as pspool,
            tc.tile_pool(name="dram", bufs=1, space="DRAM") as dpool,
        ):
            # ---- constants ----
            ident = cpool.tile([P, P], F32)
            make_identity(nc, ident[:])
            ones_row = cpool.tile([1, P], F32)
            nc.vector.memset(ones_row[:], 1.0)

            wp_sb = cpool.tile([P, KPROJ * HIDDEN], BF16)
            for k in range(KPROJ):
                nc.sync.dma_start(
                    wp_sb[:, k * HIDDEN : (k + 1) * HIDDEN], wp_in[k, :, :]
                )
            bp_sb = cpool.tile([1, HIDDEN], F32)
            nc.sync.dma_start(bp_sb[:], bp_in[:, :])

            w_scr = cpool.tile([P, 2 * XAW], F32)
            waug = []
            for li, w_in in enumerate((w1_in, w2_in)):
                wr = cpool.tile([P, 2 * XAW], F32R, name=f"w{li}")
                for k in range(2):
                    nc.sync.dma_start(w_scr[:, k * XAW : (k + 1) * XAW], w_in[k, :, :])
                nc.vector.tensor_copy(wr[:], w_scr[:])
                waug.append(wr)

            b1w2_sb = cpool.tile([1, XAW], F32)
            nc.sync.dma_start(b1w2_sb[:], b1w2_in[:, :])
            rel_sb = cpool.tile([NRELP, HIDDEN], F32)
            nc.sync.dma_start(rel_sb[:], rel_in[:, :])
            rwt_sb = cpool.tile([NRELP, NP], F32)
            nc.sync.dma_start(rwt_sb[:], rwt_in[:, :])
            worep_sb = cpool.tile([P, HIDDEN], F32)
            nc.sync.dma_start(worep_sb[:], worep_in[:, :])
            scb_sb = cpool.tile([P, 1], F32)
            nc.sync.dma_start(scb_sb[:], scb_in[:, :])
            er_sb = cpool.tile([128, SR], I16)
            nc.sync.dma_start(er_sb[:], er_in[:, :])
            eg_sb = cpool.tile([128, SG], I16)
            nc.sync.dma_start(eg_sb[:], eg_in[:, :])

            # residual h slabs + per-tile a_dst columns
            hA = hpool.tile([P, T * HIDDEN], F32)
            hB = hpool.tile([P, T * HIDDEN], F32)
            adst_all = hpool.tile([P, T * HEADS], F32)

            # DRAM bounce buffers for collectives
            h_slab = dpool.tile([NP, HIDDEN], BF16)
            xa_slab = dpool.tile([NP, GW], BF16)
            xa_slab2 = dpool.tile([NP, GW], BF16)
            h_fulls = [
                dpool.tile([NPAD, HIDDEN], BF16, addr_space="Shared", name=f"h_full{r}")
                for r in range(reps)
            ]
            xa_fulls = [
                dpool.tile([NPAD, GW], BF16, addr_space="Shared", name=f"xa_full{r}")
                for r in range(reps)
            ]
            xa_full2s = [
                dpool.tile([NPAD, GW], BF16, addr_space="Shared", name=f"xa_full2{r}")
                for r in range(reps)
            ]

            def hcols(t):
                return slice(t * HIDDEN, (t + 1) * HIDDEN)

            for rep in range(reps):
                h_full = h_fulls[rep]
                xa_full = xa_fulls[rep]
                xa_full2 = xa_full2s[rep]

                # ================= stage 1: input projection =================
                for t in range(T):
                    proj_ps = pspool.tile([P, HIDDEN], F32, tag="proj", bufs=2)
                    for k in range(KPROJ):
                        lx = lpool.tile([P, P], BF16, tag="lhsT")
                        nc.sync.dma_start(lx[:], xtt_in[t, k, :, :])
                        nc.tensor.matmul(
                            out=proj_ps[:],
                            lhsT=lx[:],
                            rhs=wp_sb[:, k * HIDDEN : (k + 1) * HIDDEN],
                            start=(k == 0),
                            stop=False,
                        )
                    nc.tensor.matmul(
                        out=proj_ps[:],
                        lhsT=ones_row[:1, :],
                        rhs=bp_sb[:1, :],
                        start=False,
                        stop=True,
                    )
                    nc.scalar.activation(
                        out=hA[:, hcols(t)],
                        in_=proj_ps[:],
                        func=mybir.ActivationFunctionType.Relu,
                    )
                    hsl = spool.tile([P, HIDDEN], BF16, tag="hsl")
                    nc.vector.tensor_copy(hsl[:], hA[:, hcols(t)])
                    if t == 0:
                        nc.sync.dma_start(
                            hsl[MROW : MROW + 1, :], mrow_in[:, 0:HIDDEN]
                        )
                    nc.sync.dma_start(h_slab[t * P : (t + 1) * P, :], hsl[:])

                if probe.get("stop_after") == "proj":
                    continue
                # ================= AllGather H =================
                if probe.get("no_collective"):
                    nc.sync.dma_start(h_full[0:NP, :], h_slab[:, :])
                else:
                    nc.gpsimd.collective_compute(
                        "AllGather",
                        mybir.AluOpType.bypass,
                        replica_groups=[list(range(NCORES))],
                        ins=[h_slab.opt()],
                        outs=[h_full.opt()],
                    )

                # ================= stage 2: relational layer =================
                for t in range(T):
                    K = K_rel[t]
                    hch = gpool.tile([P, Kmax * GW], BF16, tag="gather")
                    nc.gpsimd.dma_gather(
                        out_ap=hch[:, : K * HIDDEN].rearrange(
                            "p (k w) -> p k w", k=K
                        ),
                        in_ap=h_full[:, :],
                        idxs_ap=er_sb[:, 8 * offs_rel[t] : 8 * offs_rel[t + 1]],
                        num_idxs=K * 128,
                        num_idxs_reg=K * 128,
                        elem_size=HIDDEN,
                        single_packet=False,
                    )
                    acc = apool.tile([P, ((Kmax + 1) // 2) * MW], F32, tag="acc")
                    seg = _tree_reduce(nc, hch[:, : K * HIDDEN], acc, K, HIDDEN)
                    rel_ps = pspool.tile([P, HIDDEN], F32, tag="relps", bufs=2)
                    nc.tensor.matmul(
                        out=rel_ps[:],
                        lhsT=rwt_sb[:, t * P : (t + 1) * P],
                        rhs=rel_sb[:],
                        start=True,
                        stop=True,
                    )
                    tsum = spool.tile([P, HIDDEN], F32, tag="tsum")
                    nc.vector.tensor_add(tsum[:], seg, hA[:, hcols(t)])
                    nc.vector.tensor_add(hB[:, hcols(t)], rel_ps[:], tsum[:])

                if probe.get("stop_after") == "rel":
                    continue
                # ============ stages 3/4: GAT layers ============
                for layer in range(2):
                    hin = hB if layer == 0 else hA
                    hout = hA if layer == 0 else hB
                    wr = waug[layer]
                    slab = xa_slab if layer == 0 else xa_slab2
                    full = xa_full if layer == 0 else xa_full2

                    # ---- dense: x = h @ Waug (+ b-fold for layer 1) ----
                    for t in range(T):
                        x_ps = pspool.tile([P, XAW], F32, tag="xps", bufs=2)
                        for half in range(2):
                            tr_ps = pspool.tile([P, P], F32, tag="tr", bufs=2)
                            nc.tensor.transpose(
                                out=tr_ps[:],
                                in_=hin[
                                    :,
                                    t * HIDDEN + half * P : t * HIDDEN + (half + 1) * P,
                                ],
                                identity=ident[:],
                            )
                            ht_r = lpool.tile([P, P], F32R, tag="lhsTr")
                            nc.vector.tensor_copy(ht_r[:], tr_ps[:])
                            nc.tensor.matmul(
                                out=x_ps[:],
                                lhsT=ht_r[:],
                                rhs=wr[:, half * XAW : (half + 1) * XAW],
                                start=(half == 0),
                                stop=(half == 1 and layer == 0),
                            )
                        if layer == 1:
                            nc.tensor.matmul(
                                out=x_ps[:],
                                lhsT=ones_row[:1, :],
                                rhs=b1w2_sb[:1, :],
                                start=False,
                                stop=True,
                            )
                        xa_sb = spool.tile([P, GW], BF16, tag="xa_sb")
                        nc.vector.tensor_copy(xa_sb[:, 0:HIDDEN], x_ps[:, 0:HIDDEN])
                        nc.vector.tensor_copy(
                            xa_sb[:, HIDDEN : HIDDEN + 2 * HEADS].bitcast(F32),
                            x_ps[:, HIDDEN : HIDDEN + HEADS],
                        )
                        nc.vector.tensor_copy(
                            adst_all[:, t * HEADS : (t + 1) * HEADS],
                            x_ps[:, HIDDEN + HEADS : XAW],
                        )
                        if t == 0:
                            nc.sync.dma_start(
                                xa_sb[MROW : MROW + 1, :], mrow_in[:, :]
                            )
                        nc.sync.dma_start(slab[t * P : (t + 1) * P, :], xa_sb[:])

                    if probe.get("no_collective"):
                        nc.sync.dma_start(full[0:NP, :], slab[:, :])
                    else:
                        nc.gpsimd.collective_compute(
                            "AllGather",
                            mybir.AluOpType.bypass,
                            replica_groups=[list(range(NCORES))],
                            ins=[slab.opt()],
                            outs=[full.opt()],
                        )

                    # ---- edge stage ----
                    if probe.get("stop_after") == f"dense{layer + 1}":
                        break
                    for t in range(T):
                        K = K_gat[t]
                        xa = gpool.tile([P, Kmax * GW], BF16, tag="gather")
                        xa_v = xa[:, : K * GW].rearrange("p (k w) -> p k w", k=K)
                        nc.gpsimd.dma_gather(
                            out_ap=xa_v,
                            in_ap=full[:, :],
                            idxs_ap=eg_sb[:, 8 * offs_gat[t] : 8 * offs_gat[t + 1]],
                            num_idxs=K * 128,
                            num_idxs_reg=K * 128,
                            elem_size=GW,
                            single_packet=False,
                        )
                        af = xa[:, : K * GW].bitcast(F32).rearrange(
                            "p (k w) -> p k w", k=K
                        )
                        # alpha = a_src(f32) + a_dst  [P, K, 4]
                        alpha = spool.tile([P, Kmax * HEADS], F32, tag="alpha")
                        nc.vector.tensor_tensor(
                            out=alpha[:, : K * HEADS].rearrange(
                                "p (k h) -> p k h", k=K
                            ),
                            in0=af[:, :, HIDDEN // 2 : HIDDEN // 2 + HEADS],
                            in1=adst_all[:, t * HEADS : (t + 1) * HEADS]
                            .unsqueeze(1)
                            .to_broadcast([P, K, HEADS]),
                            op=mybir.AluOpType.add,
                        )
                        asc = spool.tile([P, Kmax * HEADS], F32, tag="asc")
                        nc.vector.tensor_scalar_mul(
                            asc[:, : K * HEADS], alpha[:, : K * HEADS], NEG_SLOPE
                        )
                        lr = spool.tile([P, Kmax * HEADS], F32, tag="lr")
                        nc.vector.tensor_tensor(
                            out=lr[:, : K * HEADS],
                            in0=alpha[:, : K * HEADS],
                            in1=asc[:, : K * HEADS],
                            op=mybir.AluOpType.max,
                        )
                        ex = spool.tile([P, Kmax * HEADS], BF16, tag="ex")
                        nc.scalar.activation(
                            out=ex[:, : K * HEADS],
                            in_=lr[:, : K * HEADS],
                            func=mybir.ActivationFunctionType.Exp,
                        )
                        ex_v = ex[:, : K * HEADS].rearrange("p (k h) -> p k h", k=K)
                        # merged [msg(256) | ex(4)] rows
                        mg = mpool.tile([P, Kmax * MW], BF16, tag="mg")
                        mg_v = mg[:, : K * MW].rearrange("p (k w) -> p k w", k=K)
                        nc.vector.tensor_tensor(
                            out=mg_v[:, :, 0:HIDDEN].rearrange(
                                "p k (h c) -> p k h c", h=HEADS
                            ),
                            in0=xa_v[:, :, 0:HIDDEN].rearrange(
                                "p k (h c) -> p k h c", h=HEADS
                            ),
                            in1=ex_v.unsqueeze(-1).to_broadcast([P, K, HEADS, CH]),
                            op=mybir.AluOpType.mult,
                        )
                        nc.vector.tensor_copy(mg_v[:, :, HIDDEN:MW], ex_v)
                        acc = apool.tile([P, ((Kmax + 1) // 2) * MW], F32, tag="acc")
                        tot = _tree_reduce(nc, mg[:, : K * MW], acc, K, MW)
                        den = spool.tile([P, HEADS], F32, tag="den")
                        nc.vector.tensor_scalar_add(den[:], tot[:, HIDDEN:MW], 1e-30)
                        dinv = spool.tile([P, HEADS], F32, tag="dinv")
                        nc.vector.reciprocal(dinv[:], den[:])
                        nc.vector.tensor_tensor(
                            out=hout[:, hcols(t)].rearrange("p (h c) -> p h c", h=HEADS),
                            in0=tot[:, 0:HIDDEN].rearrange("p (h c) -> p h c", h=HEADS),
                            in1=dinv[:].unsqueeze(-1).to_broadcast([P, HEADS, CH]),
                            op=mybir.AluOpType.mult,
                        )

                    if probe.get("stop_after") == f"gat{layer + 1}":
                        break
                if probe.get("stop_after") in ("dense1", "gat1", "dense2", "gat2"):
                    continue
                # ================= stage 5: score =================
                for t in range(T):
                    prod = spool.tile([P, HIDDEN], F32, tag="tsum")
                    nc.vector.tensor_mul(prod[:], hB[:, hcols(t)], worep_sb[:])
                    red = spool.tile([P, 1], F32, tag="red")
                    nc.vector.tensor_reduce(
                        out=red[:],
                        in_=prod[:],
                        axis=mybir.AxisListType.X,
                        op=mybir.AluOpType.add,
                    )
                    sc = spool.tile([P, 1], F32, tag="sc")
                    nc.vector.tensor_add(sc[:], red[:], scb_sb[:])
                    nc.sync.dma_start(score_out[t * P : (t + 1) * P], sc[:])

    nc.compile()
    return nc


# ---------------------------------------------------------------------------
# entry point
# ---------------------------------------------------------------------------

_CACHE = {}


def prepare(inputs, plan, probe=None):
    """Build (in_maps, nc, perm) from the full input dict + plan."""
    x = np.asarray(inputs["x"], np.float32)
    edge_index = np.asarray(inputs["edge_index"], np.int32)
    edge_type = np.asarray(inputs["edge_type"], np.int32)
    edge_weight = np.asarray(inputs["edge_weight"], np.float32)
    rel_emb = np.asarray(inputs["rel_emb"], np.float32)
    Wp = np.asarray(inputs["Wp"], np.float32)
    bp = np.asarray(inputs["bp"], np.float32)
    W1 = np.asarray(inputs["W1"], np.float32)
    W2 = np.asarray(inputs["W2"], np.float32)
    att_src1 = np.asarray(inputs["att_src1"], np.float32)
    att_dst1 = np.asarray(inputs["att_dst1"], np.float32)
    att_src2 = np.asarray(inputs["att_src2"], np.float32)
    att_dst2 = np.asarray(inputs["att_dst2"], np.float32)
    b1 = np.asarray(inputs["b1"], np.float32)
    b2 = np.asarray(inputs["b2"], np.float32)
    Wo = np.asarray(inputs["Wo"], np.float32)
    bo = np.asarray(inputs["bo"], np.float32)

    perm = plan["perm"]

    # ---- per-core dense inputs ----
    xr = np.concatenate([x[:, CODE_DIM:], CODE_WEIGHT * x[:, :CODE_DIM]], axis=1)
    xpad = np.zeros((NPAD, IN_DIM), np.float32)
    xpad[perm] = xr
    xtt = (
        xpad.reshape(NCORES, T, P, KPROJ, P).transpose(0, 1, 3, 4, 2).astype(NPBF)
    )

    w1aug = np.concatenate(
        [W1, W1 @ _asrc_mat(att_src1), W1 @ _asrc_mat(att_dst1)], axis=1
    )
    w2aug = np.concatenate(
        [W2, W2 @ _asrc_mat(att_src2), W2 @ _asrc_mat(att_dst2)], axis=1
    )
    b1w2 = (b1 @ w2aug).reshape(1, XAW).astype(np.float32)
    sc_bias = float(b2 @ Wo[:, 0] + bo[0])

    # ---- per-node relation histogram: RW[n, r] = sum of w_e over in-edges ----
    RW = np.zeros((NPAD, NRELP), np.float32)
    np.add.at(RW, (perm[edge_index[1].astype(np.int64)], edge_type), edge_weight)

    key = (plan["K_rel"], plan["K_gat"], tuple(sorted((probe or {}).items())))
    if key not in _CACHE:
        _CACHE[key] = _build_bass(
            plan["K_rel"], plan["K_gat"], plan["offs_rel"], plan["offs_gat"], probe
        )
    nc = _CACHE[key]

    common = dict(
        wp=np.ascontiguousarray(Wp.reshape(KPROJ, P, HIDDEN)).astype(NPBF),
        bp_row=bp.reshape(1, HIDDEN),
        w1aug=np.ascontiguousarray(w1aug.reshape(2, P, XAW)),
        w2aug=np.ascontiguousarray(w2aug.reshape(2, P, XAW)),
        b1w2_row=b1w2,
        rel_emb=np.concatenate(
            [rel_emb, np.zeros((NRELP - NREL, HIDDEN), np.float32)]
        ),
        wo_rep=np.ascontiguousarray(np.broadcast_to(Wo[:, 0], (P, HIDDEN))),
        sc_bias=np.full((P, 1), sc_bias, np.float32),
        mrow=_make_mrow(),
    )
    in_maps = []
    for c in range(NCORES):
        in_maps.append(
            dict(
                common,
                xtt=xtt[c],
                rwT=np.ascontiguousarray(RW[c * NP : (c + 1) * NP, :].T),
                eidx_rel=plan["eidx_rel"][c],
                eidx_gat=plan["eidx_gat"][c],
            )
        )
    return in_maps, nc, perm


def kernel(x, edge_index, **rest):
    inputs = dict(rest, x=x, edge_index=edge_index)
    edge_index = np.asarray(edge_index, np.int32)
    plan = _build_plan(edge_index)
    in_maps, nc, perm = prepare(inputs, plan)

    import os

    trace = bool(os.environ.get("GAT_TRACE"))
    res = run_bass_kernel_spmd(
        nc, in_maps, core_ids=list(range(NCORES)), trace=trace
    )
    global _LAST_RESULT
    _LAST_RESULT = res
    scores_pad = np.concatenate([r["score"] for r in res.results])
    return scores_pad[perm].astype(np.float32)


_LAST_RESULT = None


# revision 14
# speedup vs baseline: 1.3836x; 1.3352x over previous
"""Trainium2 Bass kernel for nn_GATNodeScorer (GNN message passing).

Strategy (8 NeuronCores, node-partitioned, slot-aligned edge packing):
  - Host: sort nodes by in-degree; tile (core, round) gets 125 consecutive
    sorted nodes (+3 spare slots).  All 8 cores' tiles in round j share one
    chunk count K_j = max degree in the round, so the SPMD program is
    identical across cores and per-core work is balanced.  Edges are packed
    so that slot p of chunk k holds an edge whose DESTINATION is slot p:
    segment-sum collapses to a plain reduction over chunks and per-edge
    a_dst is a direct slot lookup -- no one-hot matmuls, no transposes.
  - Device, per core (SPMD, one NEFF):
      1. input projection  h = relu(xc @ Wp + bp)
      2. AllGather H table (bf16), one dma_gather per tile fetches all
         K*128 in-edge rows (512B each) in a single SWDGE instruction
      3. relational layer  h1 = h + tree_sum_k(h[src]) + RW @ rel_emb
         (RW = per-node type/weight histogram, precomputed on host)
      4. dense x1 = h1 @ [W1 | W1@Asrc | W1@Adst]; shared-table row is
         [x bf16(256) | a_src f32 bitcast(8)| pad] = 768B; AllGather
      5. GAT layer: one dma_gather per tile; alpha = a_src(f32) + a_dst;
         ex = exp(leakyrelu(alpha)); msg = x * ex broadcast; merged
         [msg|ex] tree-reduced over chunks; normalize.  Self-loops are
         materialized as chunk 0.  Padding gathers row 127, forced to
         x=0 / a_src=-100 so ex ~ 0.
      6. repeat 4-5 for layer 2, then score = h3 @ Wo + bo

Self-contained: hardcodes all shapes; only needs numpy + the concourse repo
installed at /opt/trn_rl_repo.
"""

import sys

sys.path.insert(0, "/opt/trn_rl_repo")

import numpy as np
import ml_dtypes

import concourse.bass as bass
import concourse.bacc as bacc
import concourse.mybir as mybir
import concourse.tile as tile
from concourse.bass_utils import run_bass_kernel_spmd
from concourse.masks import make_identity

# ---- problem constants (hardcoded per contest rules) ----
N, E = 20000, 320000
IN_DIM, CODE_DIM, HIDDEN, HEADS, NREL = 896, 768, 256, 4, 5
CH = HIDDEN // HEADS
CODE_WEIGHT = 3.0
NEG_SLOPE = 0.2

NCORES = 8
P = 128
T = 20  # rounds (tiles per core)
NTILES = NCORES * T  # 160
NP = T * P  # 2560 padded nodes per core
NPAD = NTILES * P  # 20480
NPT = 125  # real nodes per tile (160*125 = 20000)
MROW = 127  # global row used for padding gathers (forced content)
KPROJ = IN_DIM // P  # 7

F32 = mybir.dt.float32
F32R = mybir.dt.float32r
BF16 = mybir.dt.bfloat16
I16 = mybir.dt.int16
NPBF = np.dtype(ml_dtypes.bfloat16)

XAW = HIDDEN + 2 * HEADS  # 264 dense output: [x | a_src | a_dst]
GW = 384  # gathered GAT table row: [x(256) | a_src f32 as 8 | pad] = 768B
MW = HIDDEN + HEADS  # 260 merged reduce row: [msg | ex]
NRELP = 6

# ---------------------------------------------------------------------------
# host-side planning
# ---------------------------------------------------------------------------


def _build_plan(edge_index):
    src = edge_index[0].astype(np.int64)
    dst = edge_index[1].astype(np.int64)
    indeg = np.bincount(dst, minlength=N)
    order = np.argsort(-indeg, kind="stable")
    ranks = np.empty(N, np.int64)
    ranks[order] = np.arange(N)
    grp = ranks // NPT
    perm = (grp % NCORES) * NP + (grp // NCORES) * P + (ranks % NPT)

    K_rel = np.array(
        [
            int(indeg[order[j * NCORES * NPT : (j + 1) * NCORES * NPT]].max())
            for j in range(T)
        ],
        np.int64,
    )
    K_gat = K_rel + 1

    pd = perm[dst]
    order_e = np.argsort(pd, kind="stable")
    sd = pd[order_e]
    ps = perm[src][order_e].astype(np.int16)
    starts = np.r_[0, np.flatnonzero(np.diff(sd)) + 1]
    kk = np.arange(E, dtype=np.int64) - np.repeat(
        starts, np.diff(np.r_[starts, E])
    )
    ec = sd // NP
    erem = sd % NP
    ej = erem // P
    es = erem % P

    offs_rel = np.r_[0, np.cumsum(K_rel)]
    offs_gat = np.r_[0, np.cumsum(K_gat)]
    SR = 8 * int(K_rel.sum())
    SG = 8 * int(K_gat.sum())
    eidx_rel = np.full((NCORES, 128, SR), MROW, np.int16)
    eidx_gat = np.full((NCORES, 128, SG), MROW, np.int16)

    for j in range(T):
        KG = int(K_gat[j])
        m = ej == j
        A = np.full((NCORES, P, KG), MROW, np.int16)
        sidx = np.arange(NPT)
        for c in range(NCORES):
            A[c, :NPT, 0] = (c * NP + j * P + sidx).astype(np.int16)
        A[ec[m], es[m], kk[m] + 1] = ps[m]
        for c in range(NCORES):
            vg = np.ascontiguousarray(A[c].T).ravel()
            img = np.ascontiguousarray(vg.reshape(-1, 16).T)
            eidx_gat[c, :, 8 * offs_gat[j] : 8 * offs_gat[j + 1]] = np.tile(
                img, (8, 1)
            )
            vr = np.ascontiguousarray(A[c, :, 1:].T).ravel()
            imgr = np.ascontiguousarray(vr.reshape(-1, 16).T)
            eidx_rel[c, :, 8 * offs_rel[j] : 8 * offs_rel[j + 1]] = np.tile(
                imgr, (8, 1)
            )

    return dict(
        perm=perm,
        K_rel=tuple(int(k) for k in K_rel),
        K_gat=tuple(int(k) for k in K_gat),
        offs_rel=tuple(int(o) for o in offs_rel),
        offs_gat=tuple(int(o) for o in offs_gat),
        eidx_rel=eidx_rel,
        eidx_gat=eidx_gat,
    )


def _make_mrow():
    """Padding-target row: x = 0, a_src (f32 bitcast at bf16 cols 256..264)
    = -100 so exp(leakyrelu(alpha)) ~ 0 for padding edges."""
    row = np.zeros((1, GW), NPBF)
    row.view(np.uint8)[0, 2 * HIDDEN : 2 * HIDDEN + 16] = (
        np.full(HEADS, -100.0, np.float32).view(np.uint8)
    )
    return row


def _asrc_mat(att):
    """[HEADS, CH] -> [HIDDEN, HEADS] block matrix so x @ A == (x*att).sum(-1)."""
    A = np.zeros((HIDDEN, HEADS), np.float32)
    for h in range(HEADS):
        A[h * CH : (h + 1) * CH, h] = att[h]
    return A


# ---------------------------------------------------------------------------
# bass program
# ---------------------------------------------------------------------------


def _tree_reduce(nc, src, acc, K, CW):
    """Sum K chunks of width CW from src (bf16 [P, K*CW]) into acc
    (f32 [P, ceil(K/2)*CW]); returns AP [P, CW] f32."""
    h = K // 2
    odd = K % 2
    if h == 0:
        nc.vector.tensor_copy(acc[:, :CW], src[:, :CW])
        return acc[:, :CW]
    nc.vector.tensor_add(acc[:, : h * CW], src[:, : h * CW], src[:, h * CW : 2 * h * CW])
    if odd:
        nc.vector.tensor_copy(
            acc[:, h * CW : (h + 1) * CW], src[:, 2 * h * CW : (2 * h + 1) * CW]
        )
        h += 1
    while h > 1:
        hh = h // 2
        odd = h % 2
        nc.vector.tensor_add(
            acc[:, : hh * CW], acc[:, : hh * CW], acc[:, hh * CW : 2 * hh * CW]
        )
        if odd:
            nc.vector.tensor_add(
                acc[:, :CW], acc[:, :CW], acc[:, 2 * hh * CW : (2 * hh + 1) * CW]
            )
        h = hh
    return acc[:, :CW]


def _build_bass(K_rel, K_gat, offs_rel, offs_gat, probe=None):
    probe = probe or {}
    reps = probe.get("reps", 1)
    Kmax = max(K_gat)
    SR = 8 * sum(K_rel)
    SG = 8 * sum(K_gat)
    nc = bacc.Bacc(
        "TRN2",
        target_bir_lowering=False,
        debug=False,
        num_devices=NCORES,
        num_swdge_queues=4,
    )

    # ---- external inputs ----
    xtt_in = nc.dram_tensor("xtt", [T, KPROJ, P, P], BF16, kind="ExternalInput")
    wp_in = nc.dram_tensor("wp", [KPROJ, P, HIDDEN], BF16, kind="ExternalInput")
    bp_in = nc.dram_tensor("bp_row", [1, HIDDEN], F32, kind="ExternalInput")
    w1_in = nc.dram_tensor("w1aug", [2, P, XAW], F32, kind="ExternalInput")
    w2_in = nc.dram_tensor("w2aug", [2, P, XAW], F32, kind="ExternalInput")
    b1w2_in = nc.dram_tensor("b1w2_row", [1, XAW], F32, kind="ExternalInput")
    rel_in = nc.dram_tensor("rel_emb", [NRELP, HIDDEN], F32, kind="ExternalInput")
    rwt_in = nc.dram_tensor("rwT", [NRELP, NP], F32, kind="ExternalInput")
    worep_in = nc.dram_tensor("wo_rep", [P, HIDDEN], F32, kind="ExternalInput")
    scb_in = nc.dram_tensor("sc_bias", [P, 1], F32, kind="ExternalInput")
    er_in = nc.dram_tensor("eidx_rel", [128, SR], I16, kind="ExternalInput")
    eg_in = nc.dram_tensor("eidx_gat", [128, SG], I16, kind="ExternalInput")
    mrow_in = nc.dram_tensor("mrow", [1, GW], BF16, kind="ExternalInput")

    score_out = nc.dram_tensor("score", [NP], F32, kind="ExternalOutput")

    with tile.TileContext(nc) as tc:
        with (
            tc.tile_pool(name="const", bufs=1) as cpool,
            tc.tile_pool(name="hres", bufs=1) as hpool,
            tc.tile_pool(name="lhsT", bufs=4) as lpool,
            tc.tile_pool(name="gather", bufs=4) as gpool,
            tc.tile_pool(name="acc", bufs=2) as apool,
            tc.tile_pool(name="small", bufs=2) as spool,
            tc.tile_pool(name="ps", bufs=1, space="PSUM") as pspool,
            tc.tile_pool(name="dram", bufs=1, space="DRAM") as dpool,
        ):
            # ---- constants ----
            ident = cpool.tile([P, P], F32)
            make_identity(nc, ident[:])
            ident_bf = cpool.tile([P, P], BF16)
            nc.vector.tensor_copy(ident_bf[:], ident[:])
            ones_row = cpool.tile([1, P], F32)
            nc.vector.memset(ones_row[:], 1.0)

            wp_sb = cpool.tile([P, KPROJ * HIDDEN], BF16)
            for k in range(KPROJ):
                nc.sync.dma_start(
                    wp_sb[:, k * HIDDEN : (k + 1) * HIDDEN], wp_in[k, :, :]
                )
            bp_sb = cpool.tile([1, HIDDEN], F32)
            nc.sync.dma_start(bp_sb[:], bp_in[:, :])

            w_scr = cpool.tile([P, 2 * XAW], F32)
            waug = []
            for li, w_in in enumerate((w1_in, w2_in)):
                wr = cpool.tile([P, 2 * XAW], F32R, name=f"w{li}")
                for k in range(2):
                    nc.sync.dma_start(w_scr[:, k * XAW : (k + 1) * XAW], w_in[k, :, :])
                nc.vector.tensor_copy(wr[:], w_scr[:])
                waug.append(wr)

            b1w2_sb = cpool.tile([1, XAW], F32)
            nc.sync.dma_start(b1w2_sb[:], b1w2_in[:, :])
            rel_sb = cpool.tile([NRELP, HIDDEN], F32)
            nc.sync.dma_start(rel_sb[:], rel_in[:, :])
            rwt_sb = cpool.tile([NRELP, NP], F32)
            nc.sync.dma_start(rwt_sb[:], rwt_in[:, :])
            worep_sb = cpool.tile([P, HIDDEN], F32)
            nc.sync.dma_start(worep_sb[:], worep_in[:, :])
            scb_sb = cpool.tile([P, 1], F32)
            nc.sync.dma_start(scb_sb[:], scb_in[:, :])
            er_sb = cpool.tile([128, SR], I16)
            nc.sync.dma_start(er_sb[:], er_in[:, :])
            eg_sb = cpool.tile([128, SG], I16)
            nc.sync.dma_start(eg_sb[:], eg_in[:, :])

            # residual h slabs + per-tile a_dst columns
            hA = hpool.tile([P, T * HIDDEN], F32)
            hB = hpool.tile([P, T * HIDDEN], F32)
            adst_all = hpool.tile([P, T * HEADS], F32)

            # DRAM bounce buffers for collectives
            h_slab = dpool.tile([NP, HIDDEN], BF16)
            xa_slab = dpool.tile([NP, GW], BF16)
            xa_slab2 = dpool.tile([NP, GW], BF16)
            h_fulls = [
                dpool.tile([NPAD, HIDDEN], BF16, addr_space="Shared", name=f"h_full{r}")
                for r in range(reps)
            ]
            xa_fulls = [
                dpool.tile([NPAD, GW], BF16, addr_space="Shared", name=f"xa_full{r}")
                for r in range(reps)
            ]
            xa_full2s = [
                dpool.tile([NPAD, GW], BF16, addr_space="Shared", name=f"xa_full2{r}")
                for r in range(reps)
            ]

            def hcols(t):
                return slice(t * HIDDEN, (t + 1) * HIDDEN)

            for rep in range(reps):
                h_full = h_fulls[rep]
                xa_full = xa_fulls[rep]
                xa_full2 = xa_full2s[rep]

                # ================= stage 1: input projection =================
                for t in range(T):
                    proj_ps = pspool.tile([P, HIDDEN], F32, tag="proj", bufs=2)
                    for k in range(KPROJ):
                        lx = lpool.tile([P, P], BF16, tag="lhsT")
                        nc.sync.dma_start(lx[:], xtt_in[t, k, :, :])
                        nc.tensor.matmul(
                            out=proj_ps[:],
                            lhsT=lx[:],
                            rhs=wp_sb[:, k * HIDDEN : (k + 1) * HIDDEN],
                            start=(k == 0),
                            stop=False,
                        )
                    nc.tensor.matmul(
                        out=proj_ps[:],
                        lhsT=ones_row[:1, :],
                        rhs=bp_sb[:1, :],
                        start=False,
                        stop=True,
                    )
                    nc.scalar.activation(
                        out=hA[:, hcols(t)],
                        in_=proj_ps[:],
                        func=mybir.ActivationFunctionType.Relu,
                    )
                    hsl = spool.tile([P, HIDDEN], BF16, tag="hsl")
                    nc.vector.tensor_copy(hsl[:], hA[:, hcols(t)])
                    if t == 0:
                        nc.sync.dma_start(
                            hsl[MROW : MROW + 1, :], mrow_in[:, 0:HIDDEN]
                        )
                    nc.sync.dma_start(h_slab[t * P : (t + 1) * P, :], hsl[:])

                if probe.get("stop_after") == "proj":
                    continue
                # ================= AllGather H =================
                if probe.get("no_collective"):
                    nc.sync.dma_start(h_full[0:NP, :], h_slab[:, :])
                else:
                    nc.gpsimd.collective_compute(
                        "AllGather",
                        mybir.AluOpType.bypass,
                        replica_groups=[list(range(NCORES))],
                        ins=[h_slab.opt()],
                        outs=[h_full.opt()],
                    )

                # ================= stage 2: relational layer =================
                for t in range(T):
                    K = K_rel[t]
                    hch = gpool.tile([P, Kmax * GW], BF16, tag="gather")
                    nc.gpsimd.dma_gather(
                        out_ap=hch[:, : K * HIDDEN].rearrange(
                            "p (k w) -> p k w", k=K
                        ),
                        in_ap=h_full[:, :],
                        idxs_ap=er_sb[:, 8 * offs_rel[t] : 8 * offs_rel[t + 1]],
                        num_idxs=K * 128,
                        num_idxs_reg=K * 128,
                        elem_size=HIDDEN,
                        single_packet=False,
                        queue_num=t % 4,
                    )
                    # segment sum on PE: identity-accumulate the K chunks,
                    # seeded with RW @ rel_emb (relation contribution)
                    seg_ps = pspool.tile([P, HIDDEN], F32, tag="relps", bufs=2)
                    nc.tensor.matmul(
                        out=seg_ps[:],
                        lhsT=rwt_sb[:, t * P : (t + 1) * P],
                        rhs=rel_sb[:],
                        start=True,
                        stop=False,
                    )
                    for k in range(K):
                        nc.tensor.matmul(
                            out=seg_ps[:],
                            lhsT=ident_bf[:],
                            rhs=hch[:, k * HIDDEN : (k + 1) * HIDDEN],
                            start=False,
                            stop=(k == K - 1),
                        )
                    nc.vector.tensor_add(hB[:, hcols(t)], seg_ps[:], hA[:, hcols(t)])

                if probe.get("stop_after") == "rel":
                    continue
                # ============ stages 3/4: GAT layers ============
                for layer in range(2):
                    hin = hB if layer == 0 else hA
                    hout = hA if layer == 0 else hB
                    wr = waug[layer]
                    slab = xa_slab if layer == 0 else xa_slab2
                    full = xa_full if layer == 0 else xa_full2

                    # ---- dense: x = h @ Waug (+ b-fold for layer 1) ----
                    for t in range(T):
                        x_ps = pspool.tile([P, XAW], F32, tag="xps", bufs=2)
                        for half in range(2):
                            tr_ps = pspool.tile([P, P], F32, tag="tr", bufs=2)
                            nc.tensor.transpose(
                                out=tr_ps[:],
                                in_=hin[
                                    :,
                                    t * HIDDEN + half * P : t * HIDDEN + (half + 1) * P,
                                ],
                                identity=ident[:],
                            )
                            ht_r = lpool.tile([P, P], F32R, tag="lhsTr")
                            nc.vector.tensor_copy(ht_r[:], tr_ps[:])
                            nc.tensor.matmul(
                                out=x_ps[:],
                                lhsT=ht_r[:],
                                rhs=wr[:, half * XAW : (half + 1) * XAW],
                                start=(half == 0),
                                stop=(half == 1 and layer == 0),
                            )
                        if layer == 1:
                            nc.tensor.matmul(
                                out=x_ps[:],
                                lhsT=ones_row[:1, :],
                                rhs=b1w2_sb[:1, :],
                                start=False,
                                stop=True,
                            )
                        xa_sb = spool.tile([P, GW], BF16, tag="xa_sb")
                        nc.vector.tensor_copy(xa_sb[:, 0:HIDDEN], x_ps[:, 0:HIDDEN])
                        nc.vector.tensor_copy(
                            xa_sb[:, HIDDEN : HIDDEN + 2 * HEADS].bitcast(F32),
                            x_ps[:, HIDDEN : HIDDEN + HEADS],
                        )
                        nc.vector.tensor_copy(
                            adst_all[:, t * HEADS : (t + 1) * HEADS],
                            x_ps[:, HIDDEN + HEADS : XAW],
                        )
                        if t == 0:
                            nc.sync.dma_start(
                                xa_sb[MROW : MROW + 1, :], mrow_in[:, :]
                            )
                        nc.sync.dma_start(slab[t * P : (t + 1) * P, :], xa_sb[:])

                    if probe.get("no_collective"):
                        nc.sync.dma_start(full[0:NP, :], slab[:, :])
                    else:
                        nc.gpsimd.collective_compute(
                            "AllGather",
                            mybir.AluOpType.bypass,
                            replica_groups=[list(range(NCORES))],
                            ins=[slab.opt()],
                            outs=[full.opt()],
                        )

                    # ---- edge stage ----
                    if probe.get("stop_after") == f"dense{layer + 1}":
                        break
                    for t in range(T):
                        K = K_gat[t]
                        xa = gpool.tile([P, Kmax * GW], BF16, tag="gather")
                        xa_v = xa[:, : K * GW].rearrange("p (k w) -> p k w", k=K)
                        nc.gpsimd.dma_gather(
                            out_ap=xa_v,
                            in_ap=full[:, :],
                            idxs_ap=eg_sb[:, 8 * offs_gat[t] : 8 * offs_gat[t + 1]],
                            num_idxs=K * 128,
                            num_idxs_reg=K * 128,
                            elem_size=GW,
                            single_packet=False,
                            queue_num=t % 4,
                        )
                        af = xa[:, : K * GW].bitcast(F32).rearrange(
                            "p (k w) -> p k w", k=K
                        )
                        # alpha = a_src(f32) + a_dst  [P, K, 4]
                        alpha = spool.tile([P, Kmax * HEADS], F32, tag="alpha")
                        nc.vector.tensor_tensor(
                            out=alpha[:, : K * HEADS].rearrange(
                                "p (k h) -> p k h", k=K
                            ),
                            in0=af[:, :, HIDDEN // 2 : HIDDEN // 2 + HEADS],
                            in1=adst_all[:, t * HEADS : (t + 1) * HEADS]
                            .unsqueeze(1)
                            .to_broadcast([P, K, HEADS]),
                            op=mybir.AluOpType.add,
                        )
                        asc = spool.tile([P, Kmax * HEADS], F32, tag="asc")
                        nc.vector.tensor_scalar_mul(
                            asc[:, : K * HEADS], alpha[:, : K * HEADS], NEG_SLOPE
                        )
                        lr = spool.tile([P, Kmax * HEADS], F32, tag="lr")
                        nc.vector.tensor_tensor(
                            out=lr[:, : K * HEADS],
                            in0=alpha[:, : K * HEADS],
                            in1=asc[:, : K * HEADS],
                            op=mybir.AluOpType.max,
                        )
                        ex = spool.tile([P, Kmax * HEADS], BF16, tag="ex")
                        nc.scalar.activation(
                            out=ex[:, : K * HEADS],
                            in_=lr[:, : K * HEADS],
                            func=mybir.ActivationFunctionType.Exp,
                        )
                        ex_v = ex[:, : K * HEADS].rearrange("p (k h) -> p k h", k=K)
                        # msg = x * ex, in place inside the gathered rows
                        nc.vector.tensor_tensor(
                            out=xa_v[:, :, 0:HIDDEN].rearrange(
                                "p k (h c) -> p k h c", h=HEADS
                            ),
                            in0=xa_v[:, :, 0:HIDDEN].rearrange(
                                "p k (h c) -> p k h c", h=HEADS
                            ),
                            in1=ex_v.unsqueeze(-1).to_broadcast([P, K, HEADS, CH]),
                            op=mybir.AluOpType.mult,
                        )
                        # numerator on PE: identity-accumulate the K chunks
                        msg_ps = pspool.tile([P, HIDDEN], F32, tag="relps", bufs=2)
                        for k in range(K):
                            nc.tensor.matmul(
                                out=msg_ps[:],
                                lhsT=ident_bf[:],
                                rhs=xa[:, k * GW : k * GW + HIDDEN],
                                start=(k == 0),
                                stop=(k == K - 1),
                            )
                        # denominator: tiny chunk tree over ex
                        acc = apool.tile([P, ((Kmax + 1) // 2) * HEADS], F32, tag="acc")
                        dsum = _tree_reduce(nc, ex[:, : K * HEADS], acc, K, HEADS)
                        den = spool.tile([P, HEADS], F32, tag="den")
                        nc.vector.tensor_scalar_add(den[:], dsum, 1e-30)
                        dinv = spool.tile([P, HEADS], F32, tag="dinv")
                        nc.vector.reciprocal(dinv[:], den[:])
                        nc.vector.tensor_tensor(
                            out=hout[:, hcols(t)].rearrange("p (h c) -> p h c", h=HEADS),
                            in0=msg_ps[:].rearrange("p (h c) -> p h c", h=HEADS),
                            in1=dinv[:].unsqueeze(-1).to_broadcast([P, HEADS, CH]),
                            op=mybir.AluOpType.mult,
                        )

                    if probe.get("stop_after") == f"gat{layer + 1}":
                        break
                if probe.get("stop_after") in ("dense1", "gat1", "dense2", "gat2"):
                    continue
                # ================= stage 5: score =================
                for t in range(T):
                    prod = spool.tile([P, HIDDEN], F32, tag="tsum")
                    nc.vector.tensor_mul(prod[:], hB[:, hcols(t)], worep_sb[:])
                    red = spool.tile([P, 1], F32, tag="red")
                    nc.vector.tensor_reduce(
                        out=red[:],
                        in_=prod[:],
                        axis=mybir.AxisListType.X,
                        op=mybir.AluOpType.add,
                    )
                    sc = spool.tile([P, 1], F32, tag="sc")
                    nc.vector.tensor_add(sc[:], red[:], scb_sb[:])
                    nc.sync.dma_start(score_out[t * P : (t + 1) * P], sc[:])

    nc.compile()
    return nc


# ---------------------------------------------------------------------------
# entry point
# ---------------------------------------------------------------------------

_CACHE = {}


def prepare(inputs, plan, probe=None):
    """Build (in_maps, nc, perm) from the full input dict + plan."""
    x = np.asarray(inputs["x"], np.float32)
    edge_index = np.asarray(inputs["edge_index"], np.int32)
    edge_type = np.asarray(inputs["edge_type"], np.int32)
    edge_weight = np.asarray(inputs["edge_weight"], np.float32)
    rel_emb = np.asarray(inputs["rel_emb"], np.float32)
    Wp = np.asarray(inputs["Wp"], np.float32)
    bp = np.asarray(inputs["bp"], np.float32)
    W1 = np.asarray(inputs["W1"], np.float32)
    W2 = np.asarray(inputs["W2"], np.float32)
    att_src1 = np.asarray(inputs["att_src1"], np.float32)
    att_dst1 = np.asarray(inputs["att_dst1"], np.float32)
    att_src2 = np.asarray(inputs["att_src2"], np.float32)
    att_dst2 = np.asarray(inputs["att_dst2"], np.float32)
    b1 = np.asarray(inputs["b1"], np.float32)
    b2 = np.asarray(inputs["b2"], np.float32)
    Wo = np.asarray(inputs["Wo"], np.float32)
    bo = np.asarray(inputs["bo"], np.float32)

    perm = plan["perm"]

    # ---- per-core dense inputs ----
    xr = np.concatenate([x[:, CODE_DIM:], CODE_WEIGHT * x[:, :CODE_DIM]], axis=1)
    xpad = np.zeros((NPAD, IN_DIM), np.float32)
    xpad[perm] = xr
    xtt = (
        xpad.reshape(NCORES, T, P, KPROJ, P).transpose(0, 1, 3, 4, 2).astype(NPBF)
    )

    w1aug = np.concatenate(
        [W1, W1 @ _asrc_mat(att_src1), W1 @ _asrc_mat(att_dst1)], axis=1
    )
    w2aug = np.concatenate(
        [W2, W2 @ _asrc_mat(att_src2), W2 @ _asrc_mat(att_dst2)], axis=1
    )
    b1w2 = (b1 @ w2aug).reshape(1, XAW).astype(np.float32)
    sc_bias = float(b2 @ Wo[:, 0] + bo[0])

    # ---- per-node relation histogram: RW[n, r] = sum of w_e over in-edges ----
    RW = np.zeros((NPAD, NRELP), np.float32)
    np.add.at(RW, (perm[edge_index[1].astype(np.int64)], edge_type), edge_weight)

    key = (plan["K_rel"], plan["K_gat"], tuple(sorted((probe or {}).items())))
    if key not in _CACHE:
        _CACHE[key] = _build_bass(
            plan["K_rel"], plan["K_gat"], plan["offs_rel"], plan["offs_gat"], probe
        )
    nc = _CACHE[key]

    common = dict(
        wp=np.ascontiguousarray(Wp.reshape(KPROJ, P, HIDDEN)).astype(NPBF),
        bp_row=bp.reshape(1, HIDDEN),
        w1aug=np.ascontiguousarray(w1aug.reshape(2, P, XAW)),
        w2aug=np.ascontiguousarray(w2aug.reshape(2, P, XAW)),
        b1w2_row=b1w2,
        rel_emb=np.concatenate(
            [rel_emb, np.zeros((NRELP - NREL, HIDDEN), np.float32)]
        ),
        wo_rep=np.ascontiguousarray(np.broadcast_to(Wo[:, 0], (P, HIDDEN))),
        sc_bias=np.full((P, 1), sc_bias, np.float32),
        mrow=_make_mrow(),
    )
    in_maps = []
    for c in range(NCORES):
        in_maps.append(
            dict(
                common,
                xtt=xtt[c],
                rwT=np.ascontiguousarray(RW[c * NP : (c + 1) * NP, :].T),
                eidx_rel=plan["eidx_rel"][c],
                eidx_gat=plan["eidx_gat"][c],
            )
        )
    return in_maps, nc, perm


def kernel(x, edge_index, **rest):
    inputs = dict(rest, x=x, edge_index=edge_index)
    edge_index = np.asarray(edge_index, np.int32)
    plan = _build_plan(edge_index)
    in_maps, nc, perm = prepare(inputs, plan)

    import os

    trace = bool(os.environ.get("GAT_TRACE"))
    res = run_bass_kernel_spmd(
        nc, in_maps, core_ids=list(range(NCORES)), trace=trace
    )
    global _LAST_RESULT
    _LAST_RESULT = res
    scores_pad = np.concatenate([r["score"] for r in res.results])
    return scores_pad[perm].astype(np.float32)


_LAST_RESULT = None


# revision 18
# speedup vs baseline: 1.5420x; 1.1145x over previous
"""Trainium2 Bass kernel for nn_GATNodeScorer (GNN message passing).

Strategy (8 NeuronCores, node-partitioned, slot-aligned edge packing):
  - Host: sort nodes by in-degree; tile (core, round) gets 125 consecutive
    sorted nodes (+3 spare slots).  All 8 cores' tiles in round j share one
    chunk count K_j = max degree in the round, so the SPMD program is
    identical across cores and per-core work is balanced.  Edges are packed
    so that slot p of chunk k holds an edge whose DESTINATION is slot p:
    segment-sum collapses to a plain reduction over chunks and per-edge
    a_dst is a direct slot lookup -- no one-hot matmuls, no transposes.
  - Device, per core (SPMD, one NEFF):
      1. input projection  h = relu(xc @ Wp + bp)
      2. AllGather H table (bf16), one dma_gather per tile fetches all
         K*128 in-edge rows (512B each) in a single SWDGE instruction
      3. relational layer  h1 = h + tree_sum_k(h[src]) + RW @ rel_emb
         (RW = per-node type/weight histogram, precomputed on host)
      4. dense x1 = h1 @ [W1 | W1@Asrc | W1@Adst]; shared-table row is
         [x bf16(256) | a_src f32 bitcast(8)| pad] = 768B; AllGather
      5. GAT layer: one dma_gather per tile; alpha = a_src(f32) + a_dst;
         ex = exp(leakyrelu(alpha)); msg = x * ex broadcast; merged
         [msg|ex] tree-reduced over chunks; normalize.  Self-loops are
         materialized as chunk 0.  Padding gathers row 127, forced to
         x=0 / a_src=-100 so ex ~ 0.
      6. repeat 4-5 for layer 2, then score = h3 @ Wo + bo

Self-contained: hardcodes all shapes; only needs numpy + the concourse repo
installed at /opt/trn_rl_repo.
"""

import sys

sys.path.insert(0, "/opt/trn_rl_repo")

import numpy as np
import ml_dtypes

import concourse.bass as bass
import concourse.bacc as bacc
import concourse.mybir as mybir
import concourse.tile as tile
from concourse.bass_utils import run_bass_kernel_spmd
from concourse.masks import make_identity

# ---- problem constants (hardcoded per contest rules) ----
N, E = 20000, 320000
IN_DIM, CODE_DIM, HIDDEN, HEADS, NREL = 896, 768, 256, 4, 5
CH = HIDDEN // HEADS
CODE_WEIGHT = 3.0
NEG_SLOPE = 0.2

NCORES = 8
P = 128
T = 20  # rounds (tiles per core)
NTILES = NCORES * T  # 160
NP = T * P  # 2560 padded nodes per core
NPAD = NTILES * P  # 20480
NPT = 125  # real nodes per tile (160*125 = 20000)
MROW = 127  # global row used for padding gathers (forced content)
KPROJ = IN_DIM // P  # 7

F32 = mybir.dt.float32
F32R = mybir.dt.float32r
BF16 = mybir.dt.bfloat16
I16 = mybir.dt.int16
NPBF = np.dtype(ml_dtypes.bfloat16)

XAW = HIDDEN + 2 * HEADS  # 264 dense output: [x | a_src | a_dst]
GW = 384  # gathered GAT table row: [x(256) | a_src f32 as 8 | pad] = 768B
MW = HIDDEN + HEADS  # 260 merged reduce row: [msg | ex]
NRELP = 6

# ---------------------------------------------------------------------------
# host-side planning
# ---------------------------------------------------------------------------


def _build_plan(edge_index):
    src = edge_index[0].astype(np.int64)
    dst = edge_index[1].astype(np.int64)
    indeg = np.bincount(dst, minlength=N)
    order = np.argsort(-indeg, kind="stable")
    ranks = np.empty(N, np.int64)
    ranks[order] = np.arange(N)
    grp = ranks // NPT
    perm = (grp % NCORES) * NP + (grp // NCORES) * P + (ranks % NPT)

    K_rel = np.array(
        [
            int(indeg[order[j * NCORES * NPT : (j + 1) * NCORES * NPT]].max())
            for j in range(T)
        ],
        np.int64,
    )
    K_gat = K_rel + 1

    pd = perm[dst]
    order_e = np.argsort(pd, kind="stable")
    sd = pd[order_e]
    ps = perm[src][order_e].astype(np.int16)
    starts = np.r_[0, np.flatnonzero(np.diff(sd)) + 1]
    kk = np.arange(E, dtype=np.int64) - np.repeat(
        starts, np.diff(np.r_[starts, E])
    )
    ec = sd // NP
    erem = sd % NP
    ej = erem // P
    es = erem % P

    offs_rel = np.r_[0, np.cumsum(K_rel)]
    offs_gat = np.r_[0, np.cumsum(K_gat)]
    SR = 8 * int(K_rel.sum())
    SG = 8 * int(K_gat.sum())
    eidx_rel = np.full((NCORES, 128, SR), MROW, np.int16)
    eidx_gat = np.full((NCORES, 128, SG), MROW, np.int16)

    for j in range(T):
        KG = int(K_gat[j])
        m = ej == j
        A = np.full((NCORES, P, KG), MROW, np.int16)
        sidx = np.arange(NPT)
        for c in range(NCORES):
            A[c, :NPT, 0] = (c * NP + j * P + sidx).astype(np.int16)
        A[ec[m], es[m], kk[m] + 1] = ps[m]
        for c in range(NCORES):
            vg = np.ascontiguousarray(A[c].T).ravel()
            img = np.ascontiguousarray(vg.reshape(-1, 16).T)
            eidx_gat[c, :, 8 * offs_gat[j] : 8 * offs_gat[j + 1]] = np.tile(
                img, (8, 1)
            )
            vr = np.ascontiguousarray(A[c, :, 1:].T).ravel()
            imgr = np.ascontiguousarray(vr.reshape(-1, 16).T)
            eidx_rel[c, :, 8 * offs_rel[j] : 8 * offs_rel[j + 1]] = np.tile(
                imgr, (8, 1)
            )

    return dict(
        perm=perm,
        K_rel=tuple(int(k) for k in K_rel),
        K_gat=tuple(int(k) for k in K_gat),
        offs_rel=tuple(int(o) for o in offs_rel),
        offs_gat=tuple(int(o) for o in offs_gat),
        eidx_rel=eidx_rel,
        eidx_gat=eidx_gat,
    )


def _make_mrow():
    """Padding-target row: x = 0, a_src (f32 bitcast at bf16 cols 256..264)
    = -100 so exp(leakyrelu(alpha)) ~ 0 for padding edges."""
    row = np.zeros((1, GW), NPBF)
    row.view(np.uint8)[0, 2 * HIDDEN : 2 * HIDDEN + 16] = (
        np.full(HEADS, -100.0, np.float32).view(np.uint8)
    )
    return row


def _asrc_mat(att):
    """[HEADS, CH] -> [HIDDEN, HEADS] block matrix so x @ A == (x*att).sum(-1)."""
    A = np.zeros((HIDDEN, HEADS), np.float32)
    for h in range(HEADS):
        A[h * CH : (h + 1) * CH, h] = att[h]
    return A


# ---------------------------------------------------------------------------
# bass program
# ---------------------------------------------------------------------------


def _tree_reduce(nc, src, acc, K, CW):
    """Sum K chunks of width CW from src (bf16 [P, K*CW]) into acc
    (f32 [P, ceil(K/2)*CW]); returns AP [P, CW] f32."""
    h = K // 2
    odd = K % 2
    if h == 0:
        nc.vector.tensor_copy(acc[:, :CW], src[:, :CW])
        return acc[:, :CW]
    nc.vector.tensor_add(acc[:, : h * CW], src[:, : h * CW], src[:, h * CW : 2 * h * CW])
    if odd:
        nc.vector.tensor_copy(
            acc[:, h * CW : (h + 1) * CW], src[:, 2 * h * CW : (2 * h + 1) * CW]
        )
        h += 1
    while h > 1:
        hh = h // 2
        odd = h % 2
        nc.vector.tensor_add(
            acc[:, : hh * CW], acc[:, : hh * CW], acc[:, hh * CW : 2 * hh * CW]
        )
        if odd:
            nc.vector.tensor_add(
                acc[:, :CW], acc[:, :CW], acc[:, 2 * hh * CW : (2 * hh + 1) * CW]
            )
        h = hh
    return acc[:, :CW]


def _build_bass(K_rel, K_gat, offs_rel, offs_gat, probe=None):
    probe = probe or {}
    reps = probe.get("reps", 1)
    Kmax = max(K_gat)
    SR = 8 * sum(K_rel)
    SG = 8 * sum(K_gat)
    nc = bacc.Bacc(
        "TRN2",
        target_bir_lowering=False,
        debug=False,
        num_devices=NCORES,
        num_swdge_queues=4,
    )

    # ---- external inputs ----
    xtt_in = nc.dram_tensor("xtt", [T, P, KPROJ * P], BF16, kind="ExternalInput")
    wp_in = nc.dram_tensor("wp", [KPROJ, P, HIDDEN], BF16, kind="ExternalInput")
    bp_in = nc.dram_tensor("bp_row", [1, HIDDEN], F32, kind="ExternalInput")
    w1_in = nc.dram_tensor("w1aug", [2, P, XAW], F32, kind="ExternalInput")
    w2_in = nc.dram_tensor("w2aug", [2, P, XAW], F32, kind="ExternalInput")
    b1w2_in = nc.dram_tensor("b1w2_row", [1, XAW], F32, kind="ExternalInput")
    rel_in = nc.dram_tensor("rel_emb", [NRELP, HIDDEN], F32, kind="ExternalInput")
    rwt_in = nc.dram_tensor("rwT", [NRELP, NP], F32, kind="ExternalInput")
    worep_in = nc.dram_tensor("wo_rep", [P, HIDDEN], F32, kind="ExternalInput")
    scb_in = nc.dram_tensor("sc_bias", [P, 1], F32, kind="ExternalInput")
    er_in = nc.dram_tensor("eidx_rel", [128, SR], I16, kind="ExternalInput")
    eg_in = nc.dram_tensor("eidx_gat", [128, SG], I16, kind="ExternalInput")
    mrow_in = nc.dram_tensor("mrow", [1, GW], BF16, kind="ExternalInput")

    score_out = nc.dram_tensor("score", [NP], F32, kind="ExternalOutput")

    with tile.TileContext(nc) as tc:
        with (
            tc.tile_pool(name="const", bufs=1) as cpool,
            tc.tile_pool(name="hres", bufs=1) as hpool,
            tc.tile_pool(name="lhsT", bufs=4) as lpool,
            tc.tile_pool(name="gather", bufs=4) as gpool,
            tc.tile_pool(name="acc", bufs=2) as apool,
            tc.tile_pool(name="small", bufs=2) as spool,
            tc.tile_pool(name="ps", bufs=1, space="PSUM") as pspool,
            tc.tile_pool(name="dram", bufs=1, space="DRAM") as dpool,
        ):
            # ---- constants ----
            ident = cpool.tile([P, P], F32)
            make_identity(nc, ident[:])
            ident_bf = cpool.tile([P, P], BF16)
            nc.vector.tensor_copy(ident_bf[:], ident[:])
            ones_row = cpool.tile([1, P], F32)
            nc.vector.memset(ones_row[:], 1.0)

            wp_sb = cpool.tile([P, KPROJ * HIDDEN], BF16)
            for k in range(KPROJ):
                nc.sync.dma_start(
                    wp_sb[:, k * HIDDEN : (k + 1) * HIDDEN], wp_in[k, :, :]
                )
            bp_sb = cpool.tile([1, HIDDEN], F32)
            nc.sync.dma_start(bp_sb[:], bp_in[:, :])

            w_scr = cpool.tile([P, 2 * XAW], F32)
            waug = []
            for li, w_in in enumerate((w1_in, w2_in)):
                wr = cpool.tile([P, 2 * XAW], F32R, name=f"w{li}")
                for k in range(2):
                    nc.sync.dma_start(w_scr[:, k * XAW : (k + 1) * XAW], w_in[k, :, :])
                nc.vector.tensor_copy(wr[:], w_scr[:])
                waug.append(wr)

            b1w2_sb = cpool.tile([1, XAW], F32)
            nc.sync.dma_start(b1w2_sb[:], b1w2_in[:, :])
            rel_sb = cpool.tile([NRELP, HIDDEN], F32)
            nc.sync.dma_start(rel_sb[:], rel_in[:, :])
            rwt_sb = cpool.tile([NRELP, NP], F32)
            nc.sync.dma_start(rwt_sb[:], rwt_in[:, :])
            worep_sb = cpool.tile([P, HIDDEN], F32)
            nc.sync.dma_start(worep_sb[:], worep_in[:, :])
            scb_sb = cpool.tile([P, 1], F32)
            nc.sync.dma_start(scb_sb[:], scb_in[:, :])
            er_sb = cpool.tile([128, SR], I16)
            nc.sync.dma_start(er_sb[:], er_in[:, :])
            eg_sb = cpool.tile([128, SG], I16)
            nc.sync.dma_start(eg_sb[:], eg_in[:, :])

            # residual h slabs + per-tile a_dst columns
            hA = hpool.tile([P, T * HIDDEN], F32)
            hB = hpool.tile([P, T * HIDDEN], F32)
            adst_all = hpool.tile([P, T * HEADS], F32)

            # DRAM bounce buffers for collectives
            h_slab = dpool.tile([NP, HIDDEN], BF16)
            xa_slab = dpool.tile([NP, GW], BF16)
            xa_slab2 = dpool.tile([NP, GW], BF16)
            h_fulls = [
                dpool.tile([NPAD, HIDDEN], BF16, addr_space="Shared", name=f"h_full{r}")
                for r in range(reps)
            ]
            xa_fulls = [
                dpool.tile([NPAD, GW], BF16, addr_space="Shared", name=f"xa_full{r}")
                for r in range(reps)
            ]
            xa_full2s = [
                dpool.tile([NPAD, GW], BF16, addr_space="Shared", name=f"xa_full2{r}")
                for r in range(reps)
            ]

            def hcols(t):
                return slice(t * HIDDEN, (t + 1) * HIDDEN)

            def emit_rel(t, h_full):
                """h1(t) = h(t) + segsum(h[src]) + RW @ rel_emb  -> hB(t)"""
                K = K_rel[t]
                hch = gpool.tile([P, Kmax * GW], BF16, tag="gather")
                nc.gpsimd.dma_gather(
                    out_ap=hch[:, : K * HIDDEN].rearrange("p (k w) -> p k w", k=K),
                    in_ap=h_full[:, :],
                    idxs_ap=er_sb[:, 8 * offs_rel[t] : 8 * offs_rel[t + 1]],
                    num_idxs=K * 128,
                    num_idxs_reg=K * 128,
                    elem_size=HIDDEN,
                    single_packet=False,
                    queue_num=t % 4,
                )
                seg_ps = pspool.tile([P, HIDDEN], F32, tag="relps", bufs=2)
                nc.tensor.matmul(
                    out=seg_ps[:],
                    lhsT=rwt_sb[:, t * P : (t + 1) * P],
                    rhs=rel_sb[:],
                    start=True,
                    stop=False,
                )
                for k in range(K):
                    nc.tensor.matmul(
                        out=seg_ps[:],
                        lhsT=ident_bf[:],
                        rhs=hch[:, k * HIDDEN : (k + 1) * HIDDEN],
                        start=False,
                        stop=(k == K - 1),
                    )
                nc.vector.tensor_add(hB[:, hcols(t)], seg_ps[:], hA[:, hcols(t)])

            def emit_dense(t, layer, hin, slab):
                """x(t) = h(t) @ Waug (+b-fold); write shared-table slab rows."""
                wr = waug[layer]
                x_ps = pspool.tile([P, XAW], F32, tag="xps", bufs=2)
                for half in range(2):
                    tr_ps = pspool.tile([P, P], F32, tag="tr", bufs=2)
                    nc.tensor.transpose(
                        out=tr_ps[:],
                        in_=hin[
                            :, t * HIDDEN + half * P : t * HIDDEN + (half + 1) * P
                        ],
                        identity=ident[:],
                    )
                    ht_r = lpool.tile([P, P], F32R, tag="lhsTr")
                    nc.vector.tensor_copy(ht_r[:], tr_ps[:])
                    nc.tensor.matmul(
                        out=x_ps[:],
                        lhsT=ht_r[:],
                        rhs=wr[:, half * XAW : (half + 1) * XAW],
                        start=(half == 0),
                        stop=(half == 1 and layer == 0),
                    )
                if layer == 1:
                    nc.tensor.matmul(
                        out=x_ps[:],
                        lhsT=ones_row[:1, :],
                        rhs=b1w2_sb[:1, :],
                        start=False,
                        stop=True,
                    )
                xa_sb = spool.tile([P, GW], BF16, tag="xa_sb")
                nc.vector.tensor_copy(xa_sb[:, 0:HIDDEN], x_ps[:, 0:HIDDEN])
                nc.vector.tensor_copy(
                    xa_sb[:, HIDDEN : HIDDEN + 2 * HEADS].bitcast(F32),
                    x_ps[:, HIDDEN : HIDDEN + HEADS],
                )
                nc.vector.tensor_copy(
                    adst_all[:, t * HEADS : (t + 1) * HEADS],
                    x_ps[:, HIDDEN + HEADS : XAW],
                )
                if t == 0:
                    nc.sync.dma_start(xa_sb[MROW : MROW + 1, :], mrow_in[:, :])
                nc.sync.dma_start(slab[t * P : (t + 1) * P, :], xa_sb[:])

            def emit_edge(t, full, hout):
                """GAT edge stage for tile t -> hout(t)."""
                K = K_gat[t]
                xa = gpool.tile([P, Kmax * GW], BF16, tag="gather")
                xa_v = xa[:, : K * GW].rearrange("p (k w) -> p k w", k=K)
                nc.gpsimd.dma_gather(
                    out_ap=xa_v,
                    in_ap=full[:, :],
                    idxs_ap=eg_sb[:, 8 * offs_gat[t] : 8 * offs_gat[t + 1]],
                    num_idxs=K * 128,
                    num_idxs_reg=K * 128,
                    elem_size=GW,
                    single_packet=False,
                    queue_num=t % 4,
                )
                af = xa[:, : K * GW].bitcast(F32).rearrange("p (k w) -> p k w", k=K)
                alpha = spool.tile([P, Kmax * HEADS], F32, tag="alpha")
                nc.vector.tensor_tensor(
                    out=alpha[:, : K * HEADS].rearrange("p (k h) -> p k h", k=K),
                    in0=af[:, :, HIDDEN // 2 : HIDDEN // 2 + HEADS],
                    in1=adst_all[:, t * HEADS : (t + 1) * HEADS]
                    .unsqueeze(1)
                    .to_broadcast([P, K, HEADS]),
                    op=mybir.AluOpType.add,
                )
                asc = spool.tile([P, Kmax * HEADS], F32, tag="asc")
                nc.vector.tensor_scalar_mul(
                    asc[:, : K * HEADS], alpha[:, : K * HEADS], NEG_SLOPE
                )
                lr = spool.tile([P, Kmax * HEADS], F32, tag="lr")
                nc.vector.tensor_tensor(
                    out=lr[:, : K * HEADS],
                    in0=alpha[:, : K * HEADS],
                    in1=asc[:, : K * HEADS],
                    op=mybir.AluOpType.max,
                )
                ex = spool.tile([P, Kmax * HEADS], BF16, tag="ex")
                nc.scalar.activation(
                    out=ex[:, : K * HEADS],
                    in_=lr[:, : K * HEADS],
                    func=mybir.ActivationFunctionType.Exp,
                )
                ex_v = ex[:, : K * HEADS].rearrange("p (k h) -> p k h", k=K)
                # msg = x * ex, in place inside the gathered rows
                nc.vector.tensor_tensor(
                    out=xa_v[:, :, 0:HIDDEN].rearrange(
                        "p k (h c) -> p k h c", h=HEADS
                    ),
                    in0=xa_v[:, :, 0:HIDDEN].rearrange(
                        "p k (h c) -> p k h c", h=HEADS
                    ),
                    in1=ex_v.unsqueeze(-1).to_broadcast([P, K, HEADS, CH]),
                    op=mybir.AluOpType.mult,
                )
                msg_ps = pspool.tile([P, HIDDEN], F32, tag="relps", bufs=2)
                for k in range(K):
                    nc.tensor.matmul(
                        out=msg_ps[:],
                        lhsT=ident_bf[:],
                        rhs=xa[:, k * GW : k * GW + HIDDEN],
                        start=(k == 0),
                        stop=(k == K - 1),
                    )
                acc = apool.tile([P, ((Kmax + 1) // 2) * HEADS], F32, tag="acc")
                dsum = _tree_reduce(nc, ex[:, : K * HEADS], acc, K, HEADS)
                den = spool.tile([P, HEADS], F32, tag="den")
                nc.vector.tensor_scalar_add(den[:], dsum, 1e-30)
                dinv = spool.tile([P, HEADS], F32, tag="dinv")
                nc.vector.reciprocal(dinv[:], den[:])
                nc.vector.tensor_tensor(
                    out=hout[:, hcols(t)].rearrange("p (h c) -> p h c", h=HEADS),
                    in0=msg_ps[:].rearrange("p (h c) -> p h c", h=HEADS),
                    in1=dinv[:].unsqueeze(-1).to_broadcast([P, HEADS, CH]),
                    op=mybir.AluOpType.mult,
                )

            def emit_score(t):
                prod = spool.tile([P, HIDDEN], F32, tag="tsum")
                nc.vector.tensor_mul(prod[:], hB[:, hcols(t)], worep_sb[:])
                red = spool.tile([P, 1], F32, tag="red")
                nc.vector.tensor_reduce(
                    out=red[:],
                    in_=prod[:],
                    axis=mybir.AxisListType.X,
                    op=mybir.AluOpType.add,
                )
                sc = spool.tile([P, 1], F32, tag="sc")
                nc.vector.tensor_add(sc[:], red[:], scb_sb[:])
                nc.sync.dma_start(score_out[t * P : (t + 1) * P], sc[:])

            def emit_ag(slab, full):
                if probe.get("no_collective"):
                    nc.sync.dma_start(full[0:NP, :], slab[:, :])
                else:
                    nc.gpsimd.collective_compute(
                        "AllGather",
                        mybir.AluOpType.bypass,
                        replica_groups=[list(range(NCORES))],
                        ins=[slab.opt()],
                        outs=[full.opt()],
                    )

            for rep in range(reps):
                h_full = h_fulls[rep]
                xa_full = xa_fulls[rep]
                xa_full2 = xa_full2s[rep]

                # ================= stage 1: input projection =================
                for t in range(T):
                    lx = lpool.tile([P, KPROJ * P], BF16, tag="lhsT")
                    nc.sync.dma_start(lx[:], xtt_in[t, :, :])
                    proj_ps = pspool.tile([P, HIDDEN], F32, tag="proj", bufs=2)
                    for k in range(KPROJ):
                        nc.tensor.matmul(
                            out=proj_ps[:],
                            lhsT=lx[:, k * P : (k + 1) * P],
                            rhs=wp_sb[:, k * HIDDEN : (k + 1) * HIDDEN],
                            start=(k == 0),
                            stop=False,
                        )
                    nc.tensor.matmul(
                        out=proj_ps[:],
                        lhsT=ones_row[:1, :],
                        rhs=bp_sb[:1, :],
                        start=False,
                        stop=True,
                    )
                    nc.scalar.activation(
                        out=hA[:, hcols(t)],
                        in_=proj_ps[:],
                        func=mybir.ActivationFunctionType.Relu,
                    )
                    hsl = spool.tile([P, HIDDEN], BF16, tag="hsl")
                    nc.vector.tensor_copy(hsl[:], hA[:, hcols(t)])
                    if t == 0:
                        nc.sync.dma_start(
                            hsl[MROW : MROW + 1, :], mrow_in[:, 0:HIDDEN]
                        )
                    nc.sync.dma_start(h_slab[t * P : (t + 1) * P, :], hsl[:])

                if probe.get("stop_after") == "proj":
                    continue
                emit_ag(h_slab, h_full)

                # ====== stage 2: relational layer fused with dense 1 ======
                for t in range(T):
                    emit_rel(t, h_full)
                    emit_dense(t, 0, hB, xa_slab)
                if probe.get("stop_after") == "rel":
                    continue
                emit_ag(xa_slab, xa_full)

                # ====== stage 3: GAT layer 1 fused with dense 2 ======
                for t in range(T):
                    emit_edge(t, xa_full, hA)
                    emit_dense(t, 1, hA, xa_slab2)
                if probe.get("stop_after") == "gat1":
                    continue
                emit_ag(xa_slab2, xa_full2)

                # ====== stage 4: GAT layer 2 fused with score ======
                for t in range(T):
                    emit_edge(t, xa_full2, hB)
                    emit_score(t)

    nc.compile()
    return nc


# ---------------------------------------------------------------------------
# entry point
# ---------------------------------------------------------------------------

_CACHE = {}


def prepare(inputs, plan, probe=None):
    """Build (in_maps, nc, perm) from the full input dict + plan."""
    x = np.asarray(inputs["x"], np.float32)
    edge_index = np.asarray(inputs["edge_index"], np.int32)
    edge_type = np.asarray(inputs["edge_type"], np.int32)
    edge_weight = np.asarray(inputs["edge_weight"], np.float32)
    rel_emb = np.asarray(inputs["rel_emb"], np.float32)
    Wp = np.asarray(inputs["Wp"], np.float32)
    bp = np.asarray(inputs["bp"], np.float32)
    W1 = np.asarray(inputs["W1"], np.float32)
    W2 = np.asarray(inputs["W2"], np.float32)
    att_src1 = np.asarray(inputs["att_src1"], np.float32)
    att_dst1 = np.asarray(inputs["att_dst1"], np.float32)
    att_src2 = np.asarray(inputs["att_src2"], np.float32)
    att_dst2 = np.asarray(inputs["att_dst2"], np.float32)
    b1 = np.asarray(inputs["b1"], np.float32)
    b2 = np.asarray(inputs["b2"], np.float32)
    Wo = np.asarray(inputs["Wo"], np.float32)
    bo = np.asarray(inputs["bo"], np.float32)

    perm = plan["perm"]

    # ---- per-core dense inputs ----
    xr = np.concatenate([x[:, CODE_DIM:], CODE_WEIGHT * x[:, :CODE_DIM]], axis=1)
    xpad = np.zeros((NPAD, IN_DIM), np.float32)
    xpad[perm] = xr
    # [C, T, P(feat-within-chunk), KPROJ*P(node)] so one DMA loads a tile's
    # whole lhsT set
    xtt = (
        xpad.reshape(NCORES, T, P, KPROJ, P)
        .transpose(0, 1, 4, 3, 2)
        .reshape(NCORES, T, P, KPROJ * P)
        .astype(NPBF)
    )

    w1aug = np.concatenate(
        [W1, W1 @ _asrc_mat(att_src1), W1 @ _asrc_mat(att_dst1)], axis=1
    )
    w2aug = np.concatenate(
        [W2, W2 @ _asrc_mat(att_src2), W2 @ _asrc_mat(att_dst2)], axis=1
    )
    b1w2 = (b1 @ w2aug).reshape(1, XAW).astype(np.float32)
    sc_bias = float(b2 @ Wo[:, 0] + bo[0])

    # ---- per-node relation histogram: RW[n, r] = sum of w_e over in-edges ----
    RW = np.zeros((NPAD, NRELP), np.float32)
    np.add.at(RW, (perm[edge_index[1].astype(np.int64)], edge_type), edge_weight)

    key = (plan["K_rel"], plan["K_gat"], tuple(sorted((probe or {}).items())))
    if key not in _CACHE:
        _CACHE[key] = _build_bass(
            plan["K_rel"], plan["K_gat"], plan["offs_rel"], plan["offs_gat"], probe
        )
    nc = _CACHE[key]

    common = dict(
        wp=np.ascontiguousarray(Wp.reshape(KPROJ, P, HIDDEN)).astype(NPBF),
        bp_row=bp.reshape(1, HIDDEN),
        w1aug=np.ascontiguousarray(w1aug.reshape(2, P, XAW)),
        w2aug=np.ascontiguousarray(w2aug.reshape(2, P, XAW)),
        b1w2_row=b1w2,
        rel_emb=np.concatenate(
            [rel_emb, np.zeros((NRELP - NREL, HIDDEN), np.float32)]
        ),
        wo_rep=np.ascontiguousarray(np.broadcast_to(Wo[:, 0], (P, HIDDEN))),
        sc_bias=np.full((P, 1), sc_bias, np.float32),
        mrow=_make_mrow(),
    )
    in_maps = []
    for c in range(NCORES):
        in_maps.append(
            dict(
                common,
                xtt=xtt[c],
                rwT=np.ascontiguousarray(RW[c * NP : (c + 1) * NP, :].T),
                eidx_rel=plan["eidx_rel"][c],
                eidx_gat=plan["eidx_gat"][c],
            )
        )
    return in_maps, nc, perm


def kernel(x, edge_index, **rest):
    inputs = dict(rest, x=x, edge_index=edge_index)
    edge_index = np.asarray(edge_index, np.int32)
    plan = _build_plan(edge_index)
    in_maps, nc, perm = prepare(inputs, plan)

    import os

    trace = bool(os.environ.get("GAT_TRACE"))
    res = run_bass_kernel_spmd(
        nc, in_maps, core_ids=list(range(NCORES)), trace=trace
    )
    global _LAST_RESULT
    _LAST_RESULT = res
    scores_pad = np.concatenate([r["score"] for r in res.results])
    return scores_pad[perm].astype(np.float32)


_LAST_RESULT = None


# revision 23
# speedup vs baseline: 1.9014x; 1.2331x over previous
"""Trainium2 Bass kernel for nn_GATNodeScorer (GNN message passing).

Strategy (8 NeuronCores, node-partitioned, slot-aligned edge packing):
  - Host: sort nodes by in-degree; tile (core, round) gets 125 consecutive
    sorted nodes (+3 spare slots).  All 8 cores' tiles in round j share one
    chunk count K_j = max degree in the round, so the SPMD program is
    identical across cores and per-core work is balanced.  Edges are packed
    so that slot p of chunk k holds an edge whose DESTINATION is slot p:
    segment-sum collapses to a plain reduction over chunks and per-edge
    a_dst is a direct slot lookup -- no one-hot matmuls, no transposes.
  - Algebraic collapse of the network tail: there is no nonlinearity after
    GAT layer 1, so layer 2 only ever sees h2 through the linear maps
    C2 = [W2*Wo per head | W2@Asrc2 | W2@Adst2] (256x12).  Distributing C2
    through layer 1's per-head softmax aggregation, the layer-1 message
    carry shrinks from 256 columns to G = x1 @ C2-blocked (4 heads x 12)
    plus a_src1: 52 f32 per node.  Layer 2's carry is y2/a2src: 8 f32.
    Both GAT tables therefore fit 256-byte gather rows, dense2 disappears,
    and the final scatter produces scores directly.
  - Device, per core (SPMD, one NEFF):
      1. input projection  h = relu(xc @ Wp + bp);  AllGather H (bf16)
      2. relational layer h1 = h + segsum(h[src]) + RW @ rel_emb via one
         dma_gather per tile (512B rows, 4 SWDGE queues) + PE
         identity-accumulate; fused per-tile with dense1 = h1 @
         [W1G | W1@As1 | W1@Ad1] (56 cols); AllGather table-1
      3. GAT layer 1: per-tile dma_gather (256B rows: G f32 + a_src f32
         bitcast in bf16 table); alpha -> exp -> tiny multiply + chunk
         tree; per-head normalize; + b1@C2 -> layer-2 carries; AllGather
      4. GAT layer 2: per-tile gather (256B rows); softmax over y2 ->
         score = sum_h num_h/den_h + (b2@Wo + bo)
  - Padding gathers row 127, forced to payload=0 / a_src=-100 so ex ~ 0.

Self-contained: hardcodes all shapes; only needs numpy + the concourse repo
installed at /opt/trn_rl_repo.
"""

import sys

sys.path.insert(0, "/opt/trn_rl_repo")

import numpy as np
import ml_dtypes

import concourse.bass as bass
import concourse.bacc as bacc
import concourse.mybir as mybir
import concourse.tile as tile
from concourse.bass_utils import run_bass_kernel_spmd
from concourse.masks import make_identity

# ---- problem constants (hardcoded per contest rules) ----
N, E = 20000, 320000
IN_DIM, CODE_DIM, HIDDEN, HEADS, NREL = 896, 768, 256, 4, 5
CH = HIDDEN // HEADS
CODE_WEIGHT = 3.0
NEG_SLOPE = 0.2

NCORES = 8
P = 128
T = 20  # rounds (tiles per core)
NTILES = NCORES * T  # 160
NP = T * P  # 2560 padded nodes per core
NPAD = NTILES * P  # 20480
NPT = 125  # real nodes per tile (160*125 = 20000)
MROW = 127  # global row used for padding gathers (forced content)
KPROJ = IN_DIM // P  # 7

F32 = mybir.dt.float32
F32R = mybir.dt.float32r
BF16 = mybir.dt.bfloat16
I16 = mybir.dt.int16
NPBF = np.dtype(ml_dtypes.bfloat16)

NC2 = 12  # C2 columns: [y2(4) | a2src(4) | a2dst(4)]
NG = HEADS * NC2  # 48 f32: per-head x1 @ C2 carry
D1W = NG + 2 * HEADS  # 56 dense-1 output cols: [G | a1src | a1dst]
TW = 128  # gathered table row: 128 bf16 = 256B
AFO = 48  # f32 col of a_src within a gathered row (bf16 cols 96..104)
NRELP = 6

# ---------------------------------------------------------------------------
# host-side planning
# ---------------------------------------------------------------------------


def _build_plan(edge_index):
    src = edge_index[0].astype(np.int64)
    dst = edge_index[1].astype(np.int64)
    indeg = np.bincount(dst, minlength=N)
    order = np.argsort(-indeg, kind="stable")
    ranks = np.empty(N, np.int64)
    ranks[order] = np.arange(N)
    grp = ranks // NPT
    perm = (grp % NCORES) * NP + (grp // NCORES) * P + (ranks % NPT)

    K_rel = np.array(
        [
            int(indeg[order[j * NCORES * NPT : (j + 1) * NCORES * NPT]].max())
            for j in range(T)
        ],
        np.int64,
    )
    K_gat = K_rel + 1

    pd = perm[dst]
    order_e = np.argsort(pd, kind="stable")
    sd = pd[order_e]
    ps = perm[src][order_e].astype(np.int16)
    starts = np.r_[0, np.flatnonzero(np.diff(sd)) + 1]
    kk = np.arange(E, dtype=np.int64) - np.repeat(
        starts, np.diff(np.r_[starts, E])
    )
    ec = sd // NP
    erem = sd % NP
    ej = erem // P
    es = erem % P

    offs_rel = np.r_[0, np.cumsum(K_rel)]
    offs_gat = np.r_[0, np.cumsum(K_gat)]
    SR = 8 * int(K_rel.sum())
    SG = 8 * int(K_gat.sum())
    eidx_rel = np.full((NCORES, 128, SR), MROW, np.int16)
    eidx_gat = np.full((NCORES, 128, SG), MROW, np.int16)

    for j in range(T):
        KG = int(K_gat[j])
        m = ej == j
        A = np.full((NCORES, P, KG), MROW, np.int16)
        sidx = np.arange(NPT)
        for c in range(NCORES):
            A[c, :NPT, 0] = (c * NP + j * P + sidx).astype(np.int16)
        A[ec[m], es[m], kk[m] + 1] = ps[m]
        for c in range(NCORES):
            vg = np.ascontiguousarray(A[c].T).ravel()
            img = np.ascontiguousarray(vg.reshape(-1, 16).T)
            eidx_gat[c, :, 8 * offs_gat[j] : 8 * offs_gat[j + 1]] = np.tile(
                img, (8, 1)
            )
            vr = np.ascontiguousarray(A[c, :, 1:].T).ravel()
            imgr = np.ascontiguousarray(vr.reshape(-1, 16).T)
            eidx_rel[c, :, 8 * offs_rel[j] : 8 * offs_rel[j + 1]] = np.tile(
                imgr, (8, 1)
            )

    return dict(
        perm=perm,
        K_rel=tuple(int(k) for k in K_rel),
        K_gat=tuple(int(k) for k in K_gat),
        offs_rel=tuple(int(o) for o in offs_rel),
        offs_gat=tuple(int(o) for o in offs_gat),
        eidx_rel=eidx_rel,
        eidx_gat=eidx_gat,
    )


def _make_mrow():
    """Padding-target rows.  Row 0: all zeros (h-table force).  Row 1:
    payload 0 with a_src (f32 at bf16 cols 96..104) = -100 so
    exp(leakyrelu(alpha)) ~ 0 for padding edges (gat-table force)."""
    rows = np.zeros((2, HIDDEN), NPBF)
    rows.view(np.uint8)[1, 4 * AFO : 4 * AFO + 16] = np.full(
        HEADS, -100.0, np.float32
    ).view(np.uint8)
    return rows


def _asrc_mat(att):
    """[HEADS, CH] -> [HIDDEN, HEADS] block matrix so x @ A == (x*att).sum(-1)."""
    A = np.zeros((HIDDEN, HEADS), np.float32)
    for h in range(HEADS):
        A[h * CH : (h + 1) * CH, h] = att[h]
    return A


# ---------------------------------------------------------------------------
# bass program
# ---------------------------------------------------------------------------


def _tree_reduce(nc, src, acc, K, CW):
    """Sum K chunks of width CW from src ([P, K*CW]) into acc
    (f32 [P, ceil(K/2)*CW]); returns AP [P, CW] f32."""
    h = K // 2
    odd = K % 2
    if h == 0:
        nc.vector.tensor_copy(acc[:, :CW], src[:, :CW])
        return acc[:, :CW]
    nc.vector.tensor_add(acc[:, : h * CW], src[:, : h * CW], src[:, h * CW : 2 * h * CW])
    if odd:
        nc.vector.tensor_copy(
            acc[:, h * CW : (h + 1) * CW], src[:, 2 * h * CW : (2 * h + 1) * CW]
        )
        h += 1
    while h > 1:
        hh = h // 2
        odd = h % 2
        nc.vector.tensor_add(
            acc[:, : hh * CW], acc[:, : hh * CW], acc[:, hh * CW : 2 * hh * CW]
        )
        if odd:
            nc.vector.tensor_add(
                acc[:, :CW], acc[:, :CW], acc[:, 2 * hh * CW : (2 * hh + 1) * CW]
            )
        h = hh
    return acc[:, :CW]


def _build_bass(K_rel, K_gat, offs_rel, offs_gat, probe=None):
    probe = probe or {}
    reps = probe.get("reps", 1)
    Kmax = max(K_gat)
    KRmax = max(K_rel)
    SR = 8 * sum(K_rel)
    SG = 8 * sum(K_gat)
    nc = bacc.Bacc(
        "TRN2",
        target_bir_lowering=False,
        debug=False,
        num_devices=NCORES,
        num_swdge_queues=4,
    )

    # ---- external inputs ----
    xtt_in = nc.dram_tensor("xtt", [T, P, KPROJ * P], BF16, kind="ExternalInput")
    wp_in = nc.dram_tensor("wp", [KPROJ, P, HIDDEN], BF16, kind="ExternalInput")
    bp_in = nc.dram_tensor("bp_row", [1, HIDDEN], F32, kind="ExternalInput")
    d1_in = nc.dram_tensor("d1aug", [2, P, D1W], F32, kind="ExternalInput")
    b1c2_in = nc.dram_tensor("b1c2_rep", [P, NC2], F32, kind="ExternalInput")
    rel_in = nc.dram_tensor("rel_emb", [NRELP, HIDDEN], F32, kind="ExternalInput")
    rwt_in = nc.dram_tensor("rwT", [NRELP, NP], F32, kind="ExternalInput")
    scb_in = nc.dram_tensor("sc_bias", [P, 1], F32, kind="ExternalInput")
    er_in = nc.dram_tensor("eidx_rel", [128, SR], I16, kind="ExternalInput")
    eg_in = nc.dram_tensor("eidx_gat", [128, SG], I16, kind="ExternalInput")
    mrow_in = nc.dram_tensor("mrow", [2, HIDDEN], BF16, kind="ExternalInput")

    score_out = nc.dram_tensor("score", [NP], F32, kind="ExternalOutput")

    with tile.TileContext(nc) as tc:
        with (
            tc.tile_pool(name="const", bufs=1) as cpool,
            tc.tile_pool(name="hres", bufs=1) as hpool,
            tc.tile_pool(name="lhsT", bufs=4) as lpool,
            tc.tile_pool(name="grel", bufs=3) as grpool,
            tc.tile_pool(name="gedge", bufs=4) as gepool,
            tc.tile_pool(name="mt", bufs=2) as mpool,
            tc.tile_pool(name="acc", bufs=2) as apool,
            tc.tile_pool(name="small", bufs=2) as spool,
            tc.tile_pool(name="ps", bufs=1, space="PSUM") as pspool,
            tc.tile_pool(name="dram", bufs=1, space="DRAM") as dpool,
        ):
            # ---- constants ----
            ident = cpool.tile([P, P], F32)
            make_identity(nc, ident[:])
            ident_bf = cpool.tile([P, P], BF16)
            nc.vector.tensor_copy(ident_bf[:], ident[:])
            ones_row = cpool.tile([1, P], F32)
            nc.vector.memset(ones_row[:], 1.0)

            wp_sb = cpool.tile([P, KPROJ * HIDDEN], BF16)
            for k in range(KPROJ):
                nc.sync.dma_start(
                    wp_sb[:, k * HIDDEN : (k + 1) * HIDDEN], wp_in[k, :, :]
                )
            bp_sb = cpool.tile([1, HIDDEN], F32)
            nc.sync.dma_start(bp_sb[:], bp_in[:, :])

            w_scr = cpool.tile([P, 2 * D1W], F32)
            d1_sb = cpool.tile([P, 2 * D1W], F32R)
            for k in range(2):
                nc.sync.dma_start(w_scr[:, k * D1W : (k + 1) * D1W], d1_in[k, :, :])
            nc.vector.tensor_copy(d1_sb[:], w_scr[:])

            b1c2_sb = cpool.tile([P, NC2], F32)
            nc.sync.dma_start(b1c2_sb[:], b1c2_in[:, :])
            rel_sb = cpool.tile([NRELP, HIDDEN], F32)
            nc.sync.dma_start(rel_sb[:], rel_in[:, :])
            rwt_sb = cpool.tile([NRELP, NP], F32)
            nc.sync.dma_start(rwt_sb[:], rwt_in[:, :])
            scb_sb = cpool.tile([P, 1], F32)
            nc.sync.dma_start(scb_sb[:], scb_in[:, :])
            er_sb = cpool.tile([128, SR], I16)
            nc.sync.dma_start(er_sb[:], er_in[:, :])
            eg_sb = cpool.tile([128, SG], I16)
            nc.sync.dma_start(eg_sb[:], eg_in[:, :])

            # residual h slabs + per-tile a_dst columns (layer 1 then 2)
            hA = hpool.tile([P, T * HIDDEN], F32)
            hB = hpool.tile([P, T * HIDDEN], F32)
            adst_all = hpool.tile([P, T * HEADS], F32)

            # DRAM bounce buffers for collectives
            h_slab = dpool.tile([NP, HIDDEN], BF16)
            t1_slab = dpool.tile([NP, TW], BF16)
            t2_slab = dpool.tile([NP, TW], BF16)
            h_fulls = [
                dpool.tile([NPAD, HIDDEN], BF16, addr_space="Shared", name=f"h_full{r}")
                for r in range(reps)
            ]
            t1_fulls = [
                dpool.tile([NPAD, TW], BF16, addr_space="Shared", name=f"t1_full{r}")
                for r in range(reps)
            ]
            t2_fulls = [
                dpool.tile([NPAD, TW], BF16, addr_space="Shared", name=f"t2_full{r}")
                for r in range(reps)
            ]

            def hcols(t):
                return slice(t * HIDDEN, (t + 1) * HIDDEN)

            def emit_rel(t, h_full):
                """h1(t) = h(t) + segsum(h[src]) + RW @ rel_emb  -> hB(t)"""
                K = K_rel[t]
                hch = grpool.tile([P, KRmax * HIDDEN], BF16, tag="grel")
                nc.gpsimd.dma_gather(
                    out_ap=hch[:, : K * HIDDEN].rearrange("p (k w) -> p k w", k=K),
                    in_ap=h_full[:, :],
                    idxs_ap=er_sb[:, 8 * offs_rel[t] : 8 * offs_rel[t + 1]],
                    num_idxs=K * 128,
                    num_idxs_reg=K * 128,
                    elem_size=HIDDEN,
                    single_packet=False,
                    queue_num=t % 4,
                )
                seg_ps = pspool.tile([P, HIDDEN], F32, tag="relps", bufs=2)
                nc.tensor.matmul(
                    out=seg_ps[:],
                    lhsT=rwt_sb[:, t * P : (t + 1) * P],
                    rhs=rel_sb[:],
                    start=True,
                    stop=False,
                )
                for k in range(K):
                    nc.tensor.matmul(
                        out=seg_ps[:],
                        lhsT=ident_bf[:],
                        rhs=hch[:, k * HIDDEN : (k + 1) * HIDDEN],
                        start=False,
                        stop=(k == K - 1),
                    )
                nc.vector.tensor_add(hB[:, hcols(t)], seg_ps[:], hA[:, hcols(t)])

            def emit_dense1(t):
                """x1aug(t) = h1(t) @ [W1G | W1@As1 | W1@Ad1]; slab-1 rows."""
                x_ps = pspool.tile([P, D1W], F32, tag="xps", bufs=2)
                for half in range(2):
                    tr_ps = pspool.tile([P, P], F32, tag="tr", bufs=2)
                    nc.tensor.transpose(
                        out=tr_ps[:],
                        in_=hB[
                            :, t * HIDDEN + half * P : t * HIDDEN + (half + 1) * P
                        ],
                        identity=ident[:],
                    )
                    ht_r = lpool.tile([P, P], F32R, tag="lhsTr")
                    nc.vector.tensor_copy(ht_r[:], tr_ps[:])
                    nc.tensor.matmul(
                        out=x_ps[:],
                        lhsT=ht_r[:],
                        rhs=d1_sb[:, half * D1W : (half + 1) * D1W],
                        start=(half == 0),
                        stop=(half == 1),
                    )
                sl = spool.tile([P, TW], BF16, tag="sl")
                slf = sl[:].bitcast(F32)
                nc.vector.tensor_copy(slf[:, 0 : NG + HEADS], x_ps[:, 0 : NG + HEADS])
                nc.vector.tensor_copy(
                    adst_all[:, t * HEADS : (t + 1) * HEADS],
                    x_ps[:, NG + HEADS : D1W],
                )
                if t == 0:
                    nc.sync.dma_start(sl[MROW : MROW + 1, :], mrow_in[1:2, 0:TW])
                nc.sync.dma_start(t1_slab[t * P : (t + 1) * P, :], sl[:])

            def emit_edge1(t, full):
                """GAT layer 1 for tile t -> layer-2 carries in slab-2."""
                K = K_gat[t]
                xa = gepool.tile([P, Kmax * TW], BF16, tag="gedge")
                nc.gpsimd.dma_gather(
                    out_ap=xa[:, : K * TW].rearrange("p (k w) -> p k w", k=K),
                    in_ap=full[:, :],
                    idxs_ap=eg_sb[:, 8 * offs_gat[t] : 8 * offs_gat[t + 1]],
                    num_idxs=K * 128,
                    num_idxs_reg=K * 128,
                    elem_size=TW,
                    single_packet=False,
                    queue_num=t % 4,
                )
                af = xa[:, : K * TW].bitcast(F32).rearrange("p (k w) -> p k w", k=K)
                alpha = spool.tile([P, Kmax * HEADS], F32, tag="alpha")
                nc.vector.tensor_tensor(
                    out=alpha[:, : K * HEADS].rearrange("p (k h) -> p k h", k=K),
                    in0=af[:, :, AFO : AFO + HEADS],
                    in1=adst_all[:, t * HEADS : (t + 1) * HEADS]
                    .unsqueeze(1)
                    .to_broadcast([P, K, HEADS]),
                    op=mybir.AluOpType.add,
                )
                asc = spool.tile([P, Kmax * HEADS], F32, tag="asc")
                nc.vector.tensor_scalar_mul(
                    asc[:, : K * HEADS], alpha[:, : K * HEADS], NEG_SLOPE
                )
                lr = spool.tile([P, Kmax * HEADS], F32, tag="lr")
                nc.vector.tensor_tensor(
                    out=lr[:, : K * HEADS],
                    in0=alpha[:, : K * HEADS],
                    in1=asc[:, : K * HEADS],
                    op=mybir.AluOpType.max,
                )
                ex = spool.tile([P, Kmax * HEADS], BF16, tag="ex")
                nc.scalar.activation(
                    out=ex[:, : K * HEADS],
                    in_=lr[:, : K * HEADS],
                    func=mybir.ActivationFunctionType.Exp,
                )
                ex_v = ex[:, : K * HEADS].rearrange("p (k h) -> p k h", k=K)
                CW = NG + HEADS  # 52: [ex*G | ex]
                mt = mpool.tile([P, Kmax * CW], F32, tag="mt")
                mt_v = mt[:, : K * CW].rearrange("p (k w) -> p k w", k=K)
                nc.vector.tensor_tensor(
                    out=mt_v[:, :, 0:NG].rearrange("p k (h j) -> p k h j", h=HEADS),
                    in0=af[:, :, 0:NG].rearrange("p k (h j) -> p k h j", h=HEADS),
                    in1=ex_v.unsqueeze(-1).to_broadcast([P, K, HEADS, NC2]),
                    op=mybir.AluOpType.mult,
                )
                nc.vector.tensor_copy(mt_v[:, :, NG:CW], ex_v)
                acc = apool.tile([P, ((Kmax + 1) // 2) * CW], F32, tag="acc")
                tot = _tree_reduce(nc, mt[:, : K * CW], acc, K, CW)
                den = spool.tile([P, HEADS], F32, tag="den")
                nc.vector.tensor_scalar_add(den[:], tot[:, NG:CW], 1e-30)
                dinv = spool.tile([P, HEADS], F32, tag="dinv")
                nc.vector.reciprocal(dinv[:], den[:])
                q = spool.tile([P, NG], F32, tag="q")
                nc.vector.tensor_tensor(
                    out=q[:].rearrange("p (h j) -> p h j", h=HEADS),
                    in0=tot[:, 0:NG].rearrange("p (h j) -> p h j", h=HEADS),
                    in1=dinv[:].unsqueeze(-1).to_broadcast([P, HEADS, NC2]),
                    op=mybir.AluOpType.mult,
                )
                # sum over the 4 layer-1 heads, then + b1@C2
                hs = spool.tile([P, 2 * NC2], F32, tag="hs")
                nc.vector.tensor_add(hs[:], q[:, 0 : 2 * NC2], q[:, 2 * NC2 : NG])
                vals = spool.tile([P, NC2], F32, tag="vals")
                nc.vector.tensor_add(vals[:], hs[:, 0:NC2], hs[:, NC2 : 2 * NC2])
                nc.vector.tensor_add(vals[:], vals[:], b1c2_sb[:])
                # layer-2 carries: [y2 f32 (cols 0:4) | ... | a2src f32 (48:52)]
                sl = spool.tile([P, TW], BF16, tag="sl")
                slf = sl[:].bitcast(F32)
                nc.vector.tensor_copy(slf[:, 0:HEADS], vals[:, 0:HEADS])
                nc.vector.tensor_copy(
                    slf[:, AFO : AFO + HEADS], vals[:, HEADS : 2 * HEADS]
                )
                nc.vector.tensor_copy(
                    adst_all[:, t * HEADS : (t + 1) * HEADS],
                    vals[:, 2 * HEADS : 3 * HEADS],
                )
                if t == 0:
                    nc.sync.dma_start(sl[MROW : MROW + 1, :], mrow_in[1:2, 0:TW])
                nc.sync.dma_start(t2_slab[t * P : (t + 1) * P, :], sl[:])

            def emit_edge2(t, full):
                """GAT layer 2 + score for tile t."""
                K = K_gat[t]
                xa = gepool.tile([P, Kmax * TW], BF16, tag="gedge")
                nc.gpsimd.dma_gather(
                    out_ap=xa[:, : K * TW].rearrange("p (k w) -> p k w", k=K),
                    in_ap=full[:, :],
                    idxs_ap=eg_sb[:, 8 * offs_gat[t] : 8 * offs_gat[t + 1]],
                    num_idxs=K * 128,
                    num_idxs_reg=K * 128,
                    elem_size=TW,
                    single_packet=False,
                    queue_num=t % 4,
                )
                af = xa[:, : K * TW].bitcast(F32).rearrange("p (k w) -> p k w", k=K)
                alpha = spool.tile([P, Kmax * HEADS], F32, tag="alpha")
                nc.vector.tensor_tensor(
                    out=alpha[:, : K * HEADS].rearrange("p (k h) -> p k h", k=K),
                    in0=af[:, :, AFO : AFO + HEADS],
                    in1=adst_all[:, t * HEADS : (t + 1) * HEADS]
                    .unsqueeze(1)
                    .to_broadcast([P, K, HEADS]),
                    op=mybir.AluOpType.add,
                )
                asc = spool.tile([P, Kmax * HEADS], F32, tag="asc")
                nc.vector.tensor_scalar_mul(
                    asc[:, : K * HEADS], alpha[:, : K * HEADS], NEG_SLOPE
                )
                lr = spool.tile([P, Kmax * HEADS], F32, tag="lr")
                nc.vector.tensor_tensor(
                    out=lr[:, : K * HEADS],
                    in0=alpha[:, : K * HEADS],
                    in1=asc[:, : K * HEADS],
                    op=mybir.AluOpType.max,
                )
                ex = spool.tile([P, Kmax * HEADS], BF16, tag="ex")
                nc.scalar.activation(
                    out=ex[:, : K * HEADS],
                    in_=lr[:, : K * HEADS],
                    func=mybir.ActivationFunctionType.Exp,
                )
                ex_v = ex[:, : K * HEADS].rearrange("p (k h) -> p k h", k=K)
                CW = 2 * HEADS  # 8: [ex*y2 | ex]
                mt = mpool.tile([P, Kmax * CW], F32, tag="mt2")
                mt_v = mt[:, : K * CW].rearrange("p (k w) -> p k w", k=K)
                nc.vector.tensor_tensor(
                    out=mt_v[:, :, 0:HEADS],
                    in0=af[:, :, 0:HEADS],
                    in1=ex_v,
                    op=mybir.AluOpType.mult,
                )
                nc.vector.tensor_copy(mt_v[:, :, HEADS:CW], ex_v)
                acc = apool.tile([P, ((Kmax + 1) // 2) * CW], F32, tag="acc2")
                tot = _tree_reduce(nc, mt[:, : K * CW], acc, K, CW)
                den = spool.tile([P, HEADS], F32, tag="den")
                nc.vector.tensor_scalar_add(den[:], tot[:, HEADS:CW], 1e-30)
                dinv = spool.tile([P, HEADS], F32, tag="dinv")
                nc.vector.reciprocal(dinv[:], den[:])
                sch = spool.tile([P, HEADS], F32, tag="sch")
                nc.vector.tensor_mul(sch[:], tot[:, 0:HEADS], dinv[:])
                red = spool.tile([P, 1], F32, tag="red")
                nc.vector.tensor_reduce(
                    out=red[:],
                    in_=sch[:],
                    axis=mybir.AxisListType.X,
                    op=mybir.AluOpType.add,
                )
                sc = spool.tile([P, 1], F32, tag="sc")
                nc.vector.tensor_add(sc[:], red[:], scb_sb[:])
                nc.sync.dma_start(score_out[t * P : (t + 1) * P], sc[:])

            def emit_ag(slab, full):
                if probe.get("no_collective"):
                    nc.sync.dma_start(full[0:NP, :], slab[:, :])
                else:
                    nc.gpsimd.collective_compute(
                        "AllGather",
                        mybir.AluOpType.bypass,
                        replica_groups=[list(range(NCORES))],
                        ins=[slab.opt()],
                        outs=[full.opt()],
                    )

            for rep in range(reps):
                h_full = h_fulls[rep]
                t1_full = t1_fulls[rep]
                t2_full = t2_fulls[rep]

                # ================= stage 1: input projection =================
                for t in range(T):
                    lx = lpool.tile([P, KPROJ * P], BF16, tag="lhsT")
                    nc.sync.dma_start(lx[:], xtt_in[t, :, :])
                    proj_ps = pspool.tile([P, HIDDEN], F32, tag="proj", bufs=2)
                    for k in range(KPROJ):
                        nc.tensor.matmul(
                            out=proj_ps[:],
                            lhsT=lx[:, k * P : (k + 1) * P],
                            rhs=wp_sb[:, k * HIDDEN : (k + 1) * HIDDEN],
                            start=(k == 0),
                            stop=False,
                        )
                    nc.tensor.matmul(
                        out=proj_ps[:],
                        lhsT=ones_row[:1, :],
                        rhs=bp_sb[:1, :],
                        start=False,
                        stop=True,
                    )
                    nc.scalar.activation(
                        out=hA[:, hcols(t)],
                        in_=proj_ps[:],
                        func=mybir.ActivationFunctionType.Relu,
                    )
                    hsl = spool.tile([P, HIDDEN], BF16, tag="hsl")
                    nc.vector.tensor_copy(hsl[:], hA[:, hcols(t)])
                    if t == 0:
                        nc.sync.dma_start(hsl[MROW : MROW + 1, :], mrow_in[0:1, :])
                    nc.sync.dma_start(h_slab[t * P : (t + 1) * P, :], hsl[:])

                if probe.get("stop_after") == "proj":
                    continue
                emit_ag(h_slab, h_full)

                # ====== stage 2: relational layer fused with dense 1 ======
                for t in range(T):
                    emit_rel(t, h_full)
                    emit_dense1(t)
                if probe.get("stop_after") == "rel":
                    continue
                emit_ag(t1_slab, t1_full)

                # ====== stage 3: GAT layer 1 (emits layer-2 carries) ======
                for t in range(T):
                    emit_edge1(t, t1_full)
                if probe.get("stop_after") == "gat1":
                    continue
                emit_ag(t2_slab, t2_full)

                # ====== stage 4: GAT layer 2 + score ======
                for t in range(T):
                    emit_edge2(t, t2_full)

    nc.compile()
    return nc


# ---------------------------------------------------------------------------
# entry point
# ---------------------------------------------------------------------------

_CACHE = {}


def prepare(inputs, plan, probe=None):
    """Build (in_maps, nc, perm) from the full input dict + plan."""
    x = np.asarray(inputs["x"], np.float32)
    edge_index = np.asarray(inputs["edge_index"], np.int32)
    edge_type = np.asarray(inputs["edge_type"], np.int32)
    edge_weight = np.asarray(inputs["edge_weight"], np.float32)
    rel_emb = np.asarray(inputs["rel_emb"], np.float32)
    Wp = np.asarray(inputs["Wp"], np.float32)
    bp = np.asarray(inputs["bp"], np.float32)
    W1 = np.asarray(inputs["W1"], np.float32)
    W2 = np.asarray(inputs["W2"], np.float32)
    att_src1 = np.asarray(inputs["att_src1"], np.float32)
    att_dst1 = np.asarray(inputs["att_dst1"], np.float32)
    att_src2 = np.asarray(inputs["att_src2"], np.float32)
    att_dst2 = np.asarray(inputs["att_dst2"], np.float32)
    b1 = np.asarray(inputs["b1"], np.float32)
    b2 = np.asarray(inputs["b2"], np.float32)
    Wo = np.asarray(inputs["Wo"], np.float32)
    bo = np.asarray(inputs["bo"], np.float32)

    perm = plan["perm"]

    # ---- per-core dense inputs ----
    xr = np.concatenate([x[:, CODE_DIM:], CODE_WEIGHT * x[:, :CODE_DIM]], axis=1)
    xpad = np.zeros((NPAD, IN_DIM), np.float32)
    xpad[perm] = xr
    # [C, T, P(feat-within-chunk), KPROJ*P(node)] so one DMA loads a tile's
    # whole lhsT set
    xtt = (
        xpad.reshape(NCORES, T, P, KPROJ, P)
        .transpose(0, 1, 4, 3, 2)
        .reshape(NCORES, T, P, KPROJ * P)
        .astype(NPBF)
    )

    # ---- algebraic collapse of the network tail ----
    # C2 = [per-head W2*Wo | W2@Asrc2 | W2@Adst2]  (256 x 12)
    W2y = np.stack(
        [W2[:, h * CH : (h + 1) * CH] @ Wo[h * CH : (h + 1) * CH, 0] for h in range(HEADS)],
        axis=1,
    )
    C2 = np.concatenate([W2y, W2 @ _asrc_mat(att_src2), W2 @ _asrc_mat(att_dst2)], axis=1)
    # G carry: W1G[:, h*12+j] = W1[:, hC:(h+1)C] @ C2[hC:(h+1)C, j]
    W1G = np.zeros((HIDDEN, NG), np.float32)
    for h in range(HEADS):
        W1G[:, h * NC2 : (h + 1) * NC2] = (
            W1[:, h * CH : (h + 1) * CH] @ C2[h * CH : (h + 1) * CH, :]
        )
    d1aug = np.concatenate(
        [W1G, W1 @ _asrc_mat(att_src1), W1 @ _asrc_mat(att_dst1)], axis=1
    )
    b1c2 = b1 @ C2  # [12]
    sc_bias = float(b2 @ Wo[:, 0] + bo[0])

    # ---- per-node relation histogram: RW[n, r] = sum of w_e over in-edges ----
    RW = np.zeros((NPAD, NRELP), np.float32)
    np.add.at(RW, (perm[edge_index[1].astype(np.int64)], edge_type), edge_weight)

    key = (plan["K_rel"], plan["K_gat"], tuple(sorted((probe or {}).items())))
    if key not in _CACHE:
        _CACHE[key] = _build_bass(
            plan["K_rel"], plan["K_gat"], plan["offs_rel"], plan["offs_gat"], probe
        )
    nc = _CACHE[key]

    common = dict(
        wp=np.ascontiguousarray(Wp.reshape(KPROJ, P, HIDDEN)).astype(NPBF),
        bp_row=bp.reshape(1, HIDDEN),
        d1aug=np.ascontiguousarray(
            np.stack([d1aug[:P], d1aug[P:]], axis=0)
        ),
        b1c2_rep=np.ascontiguousarray(np.broadcast_to(b1c2, (P, NC2))),
        rel_emb=np.concatenate(
            [rel_emb, np.zeros((NRELP - NREL, HIDDEN), np.float32)]
        ),
        sc_bias=np.full((P, 1), sc_bias, np.float32),
        mrow=_make_mrow(),
    )
    in_maps = []
    for c in range(NCORES):
        in_maps.append(
            dict(
                common,
                xtt=xtt[c],
                rwT=np.ascontiguousarray(RW[c * NP : (c + 1) * NP, :].T),
                eidx_rel=plan["eidx_rel"][c],
                eidx_gat=plan["eidx_gat"][c],
            )
        )
    return in_maps, nc, perm


def kernel(x, edge_index, **rest):
    inputs = dict(rest, x=x, edge_index=edge_index)
    edge_index = np.asarray(edge_index, np.int32)
    plan = _build_plan(edge_index)
    in_maps, nc, perm = prepare(inputs, plan)

    import os

    trace = bool(os.environ.get("GAT_TRACE"))
    res = run_bass_kernel_spmd(
        nc, in_maps, core_ids=list(range(NCORES)), trace=trace
    )
    global _LAST_RESULT
    _LAST_RESULT = res
    scores_pad = np.concatenate([r["score"] for r in res.results])
    return scores_pad[perm].astype(np.float32)


_LAST_RESULT = None


# revision 25
# speedup vs baseline: 2.6266x; 1.3814x over previous
"""Trainium2 Bass kernel for nn_GATNodeScorer (GNN message passing).

Strategy (8 NeuronCores, node-partitioned, slot-aligned edge packing):
  - Host: sort nodes by in-degree; tile (core, round) gets 125 consecutive
    sorted nodes (+3 spare slots).  All 8 cores' tiles in round j share one
    chunk count K_j = max degree in the round, so the SPMD program is
    identical across cores and per-core work is balanced.  Edges are packed
    so that slot p of chunk k holds an edge whose DESTINATION is slot p:
    segment-sum collapses to a plain reduction over chunks and per-edge
    a_dst is a direct slot lookup -- no one-hot matmuls, no transposes.
  - Algebraic collapse of the network tail: there is no nonlinearity after
    GAT layer 1, so layer 2 only ever sees h2 through the linear maps
    C2 = [W2*Wo per head | W2@Asrc2 | W2@Adst2] (256x12).  Distributing C2
    through layer 1's per-head softmax aggregation, the layer-1 message
    carry shrinks from 256 columns to G = x1 @ C2-blocked (4 heads x 12)
    plus a_src1: 52 f32 per node.  Layer 2's carry is y2/a2src: 8 f32.
    Both GAT tables therefore fit 256-byte gather rows, dense2 disappears,
    and the final scatter produces scores directly.
  - Device, per core (SPMD, one NEFF):
      1. input projection  h = relu(xc @ Wp + bp);  AllGather H (bf16)
      2. relational layer h1 = h + segsum(h[src]) + RW @ rel_emb via one
         dma_gather per tile (512B rows, 4 SWDGE queues) + PE
         identity-accumulate; fused per-tile with dense1 = h1 @
         [W1G | W1@As1 | W1@Ad1] (56 cols); AllGather table-1
      3. GAT layer 1: per-tile dma_gather (256B rows: G f32 + a_src f32
         bitcast in bf16 table); alpha -> exp -> tiny multiply + chunk
         tree; per-head normalize; + b1@C2 -> layer-2 carries; AllGather
      4. GAT layer 2: per-tile gather (256B rows); softmax over y2 ->
         score = sum_h num_h/den_h + (b2@Wo + bo)
  - Padding gathers row 127, forced to payload=0 / a_src=-100 so ex ~ 0.

Self-contained: hardcodes all shapes; only needs numpy + the concourse repo
installed at /opt/trn_rl_repo.
"""

import sys

sys.path.insert(0, "/opt/trn_rl_repo")

import numpy as np
import ml_dtypes

import concourse.bass as bass
import concourse.bacc as bacc
import concourse.mybir as mybir
import concourse.tile as tile
from concourse.bass_utils import run_bass_kernel_spmd
from concourse.masks import make_identity

# ---- problem constants (hardcoded per contest rules) ----
N, E = 20000, 320000
IN_DIM, CODE_DIM, HIDDEN, HEADS, NREL = 896, 768, 256, 4, 5
CH = HIDDEN // HEADS
CODE_WEIGHT = 3.0
NEG_SLOPE = 0.2

NCORES = 8
P = 128
T = 20  # rounds (tiles per core)
NTILES = NCORES * T  # 160
NP = T * P  # 2560 padded nodes per core
NPAD = NTILES * P  # 20480
NPT = 125  # real nodes per tile (160*125 = 20000)
MROW = 127  # global row used for padding gathers (forced content)
KPROJ = IN_DIM // P  # 7

F32 = mybir.dt.float32
F32R = mybir.dt.float32r
BF16 = mybir.dt.bfloat16
I16 = mybir.dt.int16
NPBF = np.dtype(ml_dtypes.bfloat16)

NC2 = 12  # C2 columns: [y2(4) | a2src(4) | a2dst(4)]
NG = HEADS * NC2  # 48 f32: per-head x1 @ C2 carry
D1W = NG + 2 * HEADS  # 56 dense-1 output cols: [G | a1src | a1dst]
TW = 128  # gathered table row: 128 bf16 = 256B
AFO = 48  # f32 col of a_src within a gathered row (bf16 cols 96..104)
NRELP = 6

# ---------------------------------------------------------------------------
# host-side planning
# ---------------------------------------------------------------------------


def _build_plan(edge_index):
    src = edge_index[0].astype(np.int64)
    dst = edge_index[1].astype(np.int64)
    indeg = np.bincount(dst, minlength=N)
    order = np.argsort(-indeg, kind="stable")
    ranks = np.empty(N, np.int64)
    ranks[order] = np.arange(N)
    grp = ranks // NPT
    perm = (grp % NCORES) * NP + (grp // NCORES) * P + (ranks % NPT)

    K_rel = np.array(
        [
            int(indeg[order[j * NCORES * NPT : (j + 1) * NCORES * NPT]].max())
            for j in range(T)
        ],
        np.int64,
    )
    K_gat = K_rel + 1

    pd = perm[dst]
    order_e = np.argsort(pd, kind="stable")
    sd = pd[order_e]
    ps = perm[src][order_e].astype(np.int16)
    starts = np.r_[0, np.flatnonzero(np.diff(sd)) + 1]
    kk = np.arange(E, dtype=np.int64) - np.repeat(
        starts, np.diff(np.r_[starts, E])
    )
    ec = sd // NP
    erem = sd % NP
    ej = erem // P
    es = erem % P

    offs_rel = np.r_[0, np.cumsum(K_rel)]
    offs_gat = np.r_[0, np.cumsum(K_gat)]
    SR = 8 * int(K_rel.sum())
    SG = 8 * int(K_gat.sum())
    eidx_rel = np.full((NCORES, 128, SR), MROW, np.int16)
    eidx_gat = np.full((NCORES, 128, SG), MROW, np.int16)

    for j in range(T):
        KG = int(K_gat[j])
        m = ej == j
        A = np.full((NCORES, P, KG), MROW, np.int16)
        sidx = np.arange(NPT)
        for c in range(NCORES):
            A[c, :NPT, 0] = (c * NP + j * P + sidx).astype(np.int16)
        A[ec[m], es[m], kk[m] + 1] = ps[m]
        for c in range(NCORES):
            vg = np.ascontiguousarray(A[c].T).ravel()
            img = np.ascontiguousarray(vg.reshape(-1, 16).T)
            eidx_gat[c, :, 8 * offs_gat[j] : 8 * offs_gat[j + 1]] = np.tile(
                img, (8, 1)
            )
            vr = np.ascontiguousarray(A[c, :, 1:].T).ravel()
            imgr = np.ascontiguousarray(vr.reshape(-1, 16).T)
            eidx_rel[c, :, 8 * offs_rel[j] : 8 * offs_rel[j + 1]] = np.tile(
                imgr, (8, 1)
            )

    return dict(
        perm=perm,
        K_rel=tuple(int(k) for k in K_rel),
        K_gat=tuple(int(k) for k in K_gat),
        offs_rel=tuple(int(o) for o in offs_rel),
        offs_gat=tuple(int(o) for o in offs_gat),
        eidx_rel=eidx_rel,
        eidx_gat=eidx_gat,
    )


def _make_mrow():
    """Padding-target rows.  Row 0: all zeros (h-table force).  Row 1:
    payload 0 with a_src (f32 at bf16 cols 96..104) = -100 so
    exp(leakyrelu(alpha)) ~ 0 for padding edges (gat-table force)."""
    rows = np.zeros((2, HIDDEN), NPBF)
    rows.view(np.uint8)[1, 4 * AFO : 4 * AFO + 16] = np.full(
        HEADS, -100.0, np.float32
    ).view(np.uint8)
    return rows


def _asrc_mat(att):
    """[HEADS, CH] -> [HIDDEN, HEADS] block matrix so x @ A == (x*att).sum(-1)."""
    A = np.zeros((HIDDEN, HEADS), np.float32)
    for h in range(HEADS):
        A[h * CH : (h + 1) * CH, h] = att[h]
    return A


# ---------------------------------------------------------------------------
# bass program
# ---------------------------------------------------------------------------


def _tree_reduce(nc, src, acc, K, CW):
    """Sum K chunks of width CW from src ([P, K*CW]) into acc
    (f32 [P, ceil(K/2)*CW]); returns AP [P, CW] f32."""
    h = K // 2
    odd = K % 2
    if h == 0:
        nc.vector.tensor_copy(acc[:, :CW], src[:, :CW])
        return acc[:, :CW]
    nc.vector.tensor_add(acc[:, : h * CW], src[:, : h * CW], src[:, h * CW : 2 * h * CW])
    if odd:
        nc.vector.tensor_copy(
            acc[:, h * CW : (h + 1) * CW], src[:, 2 * h * CW : (2 * h + 1) * CW]
        )
        h += 1
    while h > 1:
        hh = h // 2
        odd = h % 2
        nc.vector.tensor_add(
            acc[:, : hh * CW], acc[:, : hh * CW], acc[:, hh * CW : 2 * hh * CW]
        )
        if odd:
            nc.vector.tensor_add(
                acc[:, :CW], acc[:, :CW], acc[:, 2 * hh * CW : (2 * hh + 1) * CW]
            )
        h = hh
    return acc[:, :CW]


def _build_bass(K_rel, K_gat, offs_rel, offs_gat, probe=None):
    probe = probe or {}
    reps = probe.get("reps", 1)
    Kmax = max(K_gat)
    KRmax = max(K_rel)
    SR = 8 * sum(K_rel)
    SG = 8 * sum(K_gat)
    nc = bacc.Bacc(
        "TRN2",
        target_bir_lowering=False,
        debug=False,
        num_devices=NCORES,
        num_swdge_queues=4,
    )

    # ---- external inputs ----
    xtt_in = nc.dram_tensor("xtt", [T, P, KPROJ * P], BF16, kind="ExternalInput")
    wp_in = nc.dram_tensor("wp", [KPROJ, P, HIDDEN], BF16, kind="ExternalInput")
    bp_in = nc.dram_tensor("bp_row", [1, HIDDEN], F32, kind="ExternalInput")
    d1_in = nc.dram_tensor("d1aug", [2, P, D1W], F32, kind="ExternalInput")
    b1c2_in = nc.dram_tensor("b1c2_rep", [P, NC2], F32, kind="ExternalInput")
    rel_in = nc.dram_tensor("rel_emb", [NRELP, HIDDEN], F32, kind="ExternalInput")
    rwt_in = nc.dram_tensor("rwT", [NRELP, NP], F32, kind="ExternalInput")
    scb_in = nc.dram_tensor("sc_bias", [P, 1], F32, kind="ExternalInput")
    er_in = nc.dram_tensor("eidx_rel", [128, SR], I16, kind="ExternalInput")
    eg_in = nc.dram_tensor("eidx_gat", [128, SG], I16, kind="ExternalInput")
    mrow_in = nc.dram_tensor("mrow", [2, HIDDEN], BF16, kind="ExternalInput")

    score_out = nc.dram_tensor("score", [NP], F32, kind="ExternalOutput")

    with tile.TileContext(nc) as tc:
        with (
            tc.tile_pool(name="const", bufs=1) as cpool,
            tc.tile_pool(name="hres", bufs=1) as hpool,
            tc.tile_pool(name="lhsT", bufs=4) as lpool,
            tc.tile_pool(name="grel", bufs=4) as grpool,
            tc.tile_pool(name="gedge", bufs=4) as gepool,
            tc.tile_pool(name="mt", bufs=2) as mpool,
            tc.tile_pool(name="acc", bufs=2) as apool,
            tc.tile_pool(name="small", bufs=2) as spool,
            tc.tile_pool(name="ps", bufs=1, space="PSUM") as pspool,
            tc.tile_pool(name="dram", bufs=1, space="DRAM") as dpool,
        ):
            # ---- constants ----
            ident = cpool.tile([P, P], F32)
            make_identity(nc, ident[:])
            ident_bf = cpool.tile([P, P], BF16)
            nc.vector.tensor_copy(ident_bf[:], ident[:])
            ones_row = cpool.tile([1, P], F32)
            nc.vector.memset(ones_row[:], 1.0)

            wp_sb = cpool.tile([P, KPROJ * HIDDEN], BF16)
            for k in range(KPROJ):
                nc.sync.dma_start(
                    wp_sb[:, k * HIDDEN : (k + 1) * HIDDEN], wp_in[k, :, :]
                )
            bp_sb = cpool.tile([1, HIDDEN], F32)
            nc.sync.dma_start(bp_sb[:], bp_in[:, :])

            w_scr = cpool.tile([P, 2 * D1W], F32)
            d1_sb = cpool.tile([P, 2 * D1W], F32R)
            for k in range(2):
                nc.sync.dma_start(w_scr[:, k * D1W : (k + 1) * D1W], d1_in[k, :, :])
            nc.vector.tensor_copy(d1_sb[:], w_scr[:])

            b1c2_sb = cpool.tile([P, NC2], F32)
            nc.sync.dma_start(b1c2_sb[:], b1c2_in[:, :])
            rel_sb = cpool.tile([NRELP, HIDDEN], F32)
            nc.sync.dma_start(rel_sb[:], rel_in[:, :])
            rwt_sb = cpool.tile([NRELP, NP], F32)
            nc.sync.dma_start(rwt_sb[:], rwt_in[:, :])
            scb_sb = cpool.tile([P, 1], F32)
            nc.sync.dma_start(scb_sb[:], scb_in[:, :])
            er_sb = cpool.tile([128, SR], I16)
            nc.sync.dma_start(er_sb[:], er_in[:, :])
            eg_sb = cpool.tile([128, SG], I16)
            nc.sync.dma_start(eg_sb[:], eg_in[:, :])

            # residual h slabs + per-tile a_dst columns (layer 1 then 2)
            hA = hpool.tile([P, T * HIDDEN], F32)
            hB = hpool.tile([P, T * HIDDEN], F32)
            adst_all = hpool.tile([P, T * HEADS], F32)

            # DRAM bounce buffers for collectives
            h_slab = dpool.tile([NP, HIDDEN], BF16)
            t1_slab = dpool.tile([NP, TW], BF16)
            t2_slab = dpool.tile([NP, TW], BF16)
            h_fulls = [
                dpool.tile([NPAD, HIDDEN], BF16, addr_space="Shared", name=f"h_full{r}")
                for r in range(reps)
            ]
            t1_fulls = [
                dpool.tile([NPAD, TW], BF16, addr_space="Shared", name=f"t1_full{r}")
                for r in range(reps)
            ]
            t2_fulls = [
                dpool.tile([NPAD, TW], BF16, addr_space="Shared", name=f"t2_full{r}")
                for r in range(reps)
            ]

            def hcols(t):
                return slice(t * HIDDEN, (t + 1) * HIDDEN)

            def emit_gather2(xa, full, idx_sb, off, K, W, t):
                """One tile's gather as two half-gathers on two SWDGE queues."""
                K1 = (K + 1) // 2
                for i, (a, b) in enumerate(((0, K1), (K1, K))):
                    if b <= a:
                        continue
                    nc.gpsimd.dma_gather(
                        out_ap=xa[:, a * W : b * W].rearrange(
                            "p (k w) -> p k w", k=b - a
                        ),
                        in_ap=full[:, :],
                        idxs_ap=idx_sb[:, 8 * (off + a) : 8 * (off + b)],
                        num_idxs=(b - a) * 128,
                        num_idxs_reg=(b - a) * 128,
                        elem_size=W,
                        single_packet=False,
                        queue_num=(2 * t + i) % 4,
                    )

            def emit_rel(t, h_full):
                """h1(t) = h(t) + segsum(h[src]) + RW @ rel_emb  -> hB(t)"""
                K = K_rel[t]
                hch = grpool.tile([P, KRmax * HIDDEN], BF16, tag="grel")
                emit_gather2(hch, h_full, er_sb, offs_rel[t], K, HIDDEN, t)
                seg_ps = pspool.tile([P, HIDDEN], F32, tag="relps", bufs=2)
                nc.tensor.matmul(
                    out=seg_ps[:],
                    lhsT=rwt_sb[:, t * P : (t + 1) * P],
                    rhs=rel_sb[:],
                    start=True,
                    stop=False,
                )
                for k in range(K):
                    nc.tensor.matmul(
                        out=seg_ps[:],
                        lhsT=ident_bf[:],
                        rhs=hch[:, k * HIDDEN : (k + 1) * HIDDEN],
                        start=False,
                        stop=(k == K - 1),
                    )
                nc.vector.tensor_add(hB[:, hcols(t)], seg_ps[:], hA[:, hcols(t)])

            def emit_dense1(t):
                """x1aug(t) = h1(t) @ [W1G | W1@As1 | W1@Ad1]; slab-1 rows."""
                x_ps = pspool.tile([P, D1W], F32, tag="xps", bufs=2)
                for half in range(2):
                    tr_ps = pspool.tile([P, P], F32, tag="tr", bufs=2)
                    nc.tensor.transpose(
                        out=tr_ps[:],
                        in_=hB[
                            :, t * HIDDEN + half * P : t * HIDDEN + (half + 1) * P
                        ],
                        identity=ident[:],
                    )
                    ht_r = lpool.tile([P, P], F32R, tag="lhsTr")
                    nc.vector.tensor_copy(ht_r[:], tr_ps[:])
                    nc.tensor.matmul(
                        out=x_ps[:],
                        lhsT=ht_r[:],
                        rhs=d1_sb[:, half * D1W : (half + 1) * D1W],
                        start=(half == 0),
                        stop=(half == 1),
                    )
                sl = spool.tile([P, TW], BF16, tag="sl")
                slf = sl[:].bitcast(F32)
                nc.vector.tensor_copy(slf[:, 0 : NG + HEADS], x_ps[:, 0 : NG + HEADS])
                nc.vector.tensor_copy(
                    adst_all[:, t * HEADS : (t + 1) * HEADS],
                    x_ps[:, NG + HEADS : D1W],
                )
                if t == 0:
                    nc.sync.dma_start(sl[MROW : MROW + 1, :], mrow_in[1:2, 0:TW])
                nc.sync.dma_start(t1_slab[t * P : (t + 1) * P, :], sl[:])

            def emit_edge1(t, full):
                """GAT layer 1 for tile t -> layer-2 carries in slab-2."""
                K = K_gat[t]
                xa = gepool.tile([P, Kmax * TW], BF16, tag="gedge")
                emit_gather2(xa, full, eg_sb, offs_gat[t], K, TW, t)
                af = xa[:, : K * TW].bitcast(F32).rearrange("p (k w) -> p k w", k=K)
                alpha = spool.tile([P, Kmax * HEADS], F32, tag="alpha")
                nc.vector.tensor_tensor(
                    out=alpha[:, : K * HEADS].rearrange("p (k h) -> p k h", k=K),
                    in0=af[:, :, AFO : AFO + HEADS],
                    in1=adst_all[:, t * HEADS : (t + 1) * HEADS]
                    .unsqueeze(1)
                    .to_broadcast([P, K, HEADS]),
                    op=mybir.AluOpType.add,
                )
                asc = spool.tile([P, Kmax * HEADS], F32, tag="asc")
                nc.vector.tensor_scalar_mul(
                    asc[:, : K * HEADS], alpha[:, : K * HEADS], NEG_SLOPE
                )
                lr = spool.tile([P, Kmax * HEADS], F32, tag="lr")
                nc.vector.tensor_tensor(
                    out=lr[:, : K * HEADS],
                    in0=alpha[:, : K * HEADS],
                    in1=asc[:, : K * HEADS],
                    op=mybir.AluOpType.max,
                )
                ex = spool.tile([P, Kmax * HEADS], BF16, tag="ex")
                nc.scalar.activation(
                    out=ex[:, : K * HEADS],
                    in_=lr[:, : K * HEADS],
                    func=mybir.ActivationFunctionType.Exp,
                )
                ex_v = ex[:, : K * HEADS].rearrange("p (k h) -> p k h", k=K)
                CW = NG + HEADS  # 52: [ex*G | ex]
                mt = mpool.tile([P, Kmax * CW], F32, tag="mt")
                mt_v = mt[:, : K * CW].rearrange("p (k w) -> p k w", k=K)
                nc.vector.tensor_tensor(
                    out=mt_v[:, :, 0:NG].rearrange("p k (h j) -> p k h j", h=HEADS),
                    in0=af[:, :, 0:NG].rearrange("p k (h j) -> p k h j", h=HEADS),
                    in1=ex_v.unsqueeze(-1).to_broadcast([P, K, HEADS, NC2]),
                    op=mybir.AluOpType.mult,
                )
                nc.vector.tensor_copy(mt_v[:, :, NG:CW], ex_v)
                acc = apool.tile([P, ((Kmax + 1) // 2) * CW], F32, tag="acc")
                tot = _tree_reduce(nc, mt[:, : K * CW], acc, K, CW)
                den = spool.tile([P, HEADS], F32, tag="den")
                nc.vector.tensor_scalar_add(den[:], tot[:, NG:CW], 1e-30)
                dinv = spool.tile([P, HEADS], F32, tag="dinv")
                nc.vector.reciprocal(dinv[:], den[:])
                q = spool.tile([P, NG], F32, tag="q")
                nc.vector.tensor_tensor(
                    out=q[:].rearrange("p (h j) -> p h j", h=HEADS),
                    in0=tot[:, 0:NG].rearrange("p (h j) -> p h j", h=HEADS),
                    in1=dinv[:].unsqueeze(-1).to_broadcast([P, HEADS, NC2]),
                    op=mybir.AluOpType.mult,
                )
                # sum over the 4 layer-1 heads, then + b1@C2
                hs = spool.tile([P, 2 * NC2], F32, tag="hs")
                nc.vector.tensor_add(hs[:], q[:, 0 : 2 * NC2], q[:, 2 * NC2 : NG])
                vals = spool.tile([P, NC2], F32, tag="vals")
                nc.vector.tensor_add(vals[:], hs[:, 0:NC2], hs[:, NC2 : 2 * NC2])
                nc.vector.tensor_add(vals[:], vals[:], b1c2_sb[:])
                # layer-2 carries: [y2 f32 (cols 0:4) | ... | a2src f32 (48:52)]
                sl = spool.tile([P, TW], BF16, tag="sl")
                slf = sl[:].bitcast(F32)
                nc.vector.tensor_copy(slf[:, 0:HEADS], vals[:, 0:HEADS])
                nc.vector.tensor_copy(
                    slf[:, AFO : AFO + HEADS], vals[:, HEADS : 2 * HEADS]
                )
                nc.vector.tensor_copy(
                    adst_all[:, t * HEADS : (t + 1) * HEADS],
                    vals[:, 2 * HEADS : 3 * HEADS],
                )
                if t == 0:
                    nc.sync.dma_start(sl[MROW : MROW + 1, :], mrow_in[1:2, 0:TW])
                nc.sync.dma_start(t2_slab[t * P : (t + 1) * P, :], sl[:])

            def emit_edge2(t, full):
                """GAT layer 2 + score for tile t."""
                K = K_gat[t]
                xa = gepool.tile([P, Kmax * TW], BF16, tag="gedge")
                emit_gather2(xa, full, eg_sb, offs_gat[t], K, TW, t)
                af = xa[:, : K * TW].bitcast(F32).rearrange("p (k w) -> p k w", k=K)
                alpha = spool.tile([P, Kmax * HEADS], F32, tag="alpha")
                nc.vector.tensor_tensor(
                    out=alpha[:, : K * HEADS].rearrange("p (k h) -> p k h", k=K),
                    in0=af[:, :, AFO : AFO + HEADS],
                    in1=adst_all[:, t * HEADS : (t + 1) * HEADS]
                    .unsqueeze(1)
                    .to_broadcast([P, K, HEADS]),
                    op=mybir.AluOpType.add,
                )
                asc = spool.tile([P, Kmax * HEADS], F32, tag="asc")
                nc.vector.tensor_scalar_mul(
                    asc[:, : K * HEADS], alpha[:, : K * HEADS], NEG_SLOPE
                )
                lr = spool.tile([P, Kmax * HEADS], F32, tag="lr")
                nc.vector.tensor_tensor(
                    out=lr[:, : K * HEADS],
                    in0=alpha[:, : K * HEADS],
                    in1=asc[:, : K * HEADS],
                    op=mybir.AluOpType.max,
                )
                ex = spool.tile([P, Kmax * HEADS], BF16, tag="ex")
                nc.scalar.activation(
                    out=ex[:, : K * HEADS],
                    in_=lr[:, : K * HEADS],
                    func=mybir.ActivationFunctionType.Exp,
                )
                ex_v = ex[:, : K * HEADS].rearrange("p (k h) -> p k h", k=K)
                CW = 2 * HEADS  # 8: [ex*y2 | ex]
                mt = mpool.tile([P, Kmax * CW], F32, tag="mt2")
                mt_v = mt[:, : K * CW].rearrange("p (k w) -> p k w", k=K)
                nc.vector.tensor_tensor(
                    out=mt_v[:, :, 0:HEADS],
                    in0=af[:, :, 0:HEADS],
                    in1=ex_v,
                    op=mybir.AluOpType.mult,
                )
                nc.vector.tensor_copy(mt_v[:, :, HEADS:CW], ex_v)
                acc = apool.tile([P, ((Kmax + 1) // 2) * CW], F32, tag="acc2")
                tot = _tree_reduce(nc, mt[:, : K * CW], acc, K, CW)
                den = spool.tile([P, HEADS], F32, tag="den")
                nc.vector.tensor_scalar_add(den[:], tot[:, HEADS:CW], 1e-30)
                dinv = spool.tile([P, HEADS], F32, tag="dinv")
                nc.vector.reciprocal(dinv[:], den[:])
                sch = spool.tile([P, HEADS], F32, tag="sch")
                nc.vector.tensor_mul(sch[:], tot[:, 0:HEADS], dinv[:])
                red = spool.tile([P, 1], F32, tag="red")
                nc.vector.tensor_reduce(
                    out=red[:],
                    in_=sch[:],
                    axis=mybir.AxisListType.X,
                    op=mybir.AluOpType.add,
                )
                sc = spool.tile([P, 1], F32, tag="sc")
                nc.vector.tensor_add(sc[:], red[:], scb_sb[:])
                nc.sync.dma_start(score_out[t * P : (t + 1) * P], sc[:])

            def emit_ag(slab, full):
                if probe.get("no_collective"):
                    nc.sync.dma_start(full[0:NP, :], slab[:, :])
                else:
                    nc.gpsimd.collective_compute(
                        "AllGather",
                        mybir.AluOpType.bypass,
                        replica_groups=[list(range(NCORES))],
                        ins=[slab.opt()],
                        outs=[full.opt()],
                    )

            for rep in range(reps):
                h_full = h_fulls[rep]
                t1_full = t1_fulls[rep]
                t2_full = t2_fulls[rep]

                # ================= stage 1: input projection =================
                for t in range(T):
                    lx = lpool.tile([P, KPROJ * P], BF16, tag="lhsT")
                    nc.sync.dma_start(lx[:], xtt_in[t, :, :])
                    proj_ps = pspool.tile([P, HIDDEN], F32, tag="proj", bufs=2)
                    for k in range(KPROJ):
                        nc.tensor.matmul(
                            out=proj_ps[:],
                            lhsT=lx[:, k * P : (k + 1) * P],
                            rhs=wp_sb[:, k * HIDDEN : (k + 1) * HIDDEN],
                            start=(k == 0),
                            stop=False,
                        )
                    nc.tensor.matmul(
                        out=proj_ps[:],
                        lhsT=ones_row[:1, :],
                        rhs=bp_sb[:1, :],
                        start=False,
                        stop=True,
                    )
                    nc.scalar.activation(
                        out=hA[:, hcols(t)],
                        in_=proj_ps[:],
                        func=mybir.ActivationFunctionType.Relu,
                    )
                    hsl = spool.tile([P, HIDDEN], BF16, tag="hsl")
                    nc.vector.tensor_copy(hsl[:], hA[:, hcols(t)])
                    if t == 0:
                        nc.sync.dma_start(hsl[MROW : MROW + 1, :], mrow_in[0:1, :])
                    nc.sync.dma_start(h_slab[t * P : (t + 1) * P, :], hsl[:])

                if probe.get("stop_after") == "proj":
                    continue
                emit_ag(h_slab, h_full)

                # ====== stage 2: relational layer fused with dense 1 ======
                for t in range(T):
                    emit_rel(t, h_full)
                    emit_dense1(t)
                if probe.get("stop_after") == "rel":
                    continue
                emit_ag(t1_slab, t1_full)

                # ====== stage 3: GAT layer 1 (emits layer-2 carries) ======
                for t in range(T):
                    emit_edge1(t, t1_full)
                if probe.get("stop_after") == "gat1":
                    continue
                emit_ag(t2_slab, t2_full)

                # ====== stage 4: GAT layer 2 + score ======
                for t in range(T):
                    emit_edge2(t, t2_full)

    nc.compile()
    return nc


# ---------------------------------------------------------------------------
# entry point
# ---------------------------------------------------------------------------

_CACHE = {}


def prepare(inputs, plan, probe=None):
    """Build (in_maps, nc, perm) from the full input dict + plan."""
    x = np.asarray(inputs["x"], np.float32)
    edge_index = np.asarray(inputs["edge_index"], np.int32)
    edge_type = np.asarray(inputs["edge_type"], np.int32)
    edge_weight = np.asarray(inputs["edge_weight"], np.float32)
    rel_emb = np.asarray(inputs["rel_emb"], np.float32)
    Wp = np.asarray(inputs["Wp"], np.float32)
    bp = np.asarray(inputs["bp"], np.float32)
    W1 = np.asarray(inputs["W1"], np.float32)
    W2 = np.asarray(inputs["W2"], np.float32)
    att_src1 = np.asarray(inputs["att_src1"], np.float32)
    att_dst1 = np.asarray(inputs["att_dst1"], np.float32)
    att_src2 = np.asarray(inputs["att_src2"], np.float32)
    att_dst2 = np.asarray(inputs["att_dst2"], np.float32)
    b1 = np.asarray(inputs["b1"], np.float32)
    b2 = np.asarray(inputs["b2"], np.float32)
    Wo = np.asarray(inputs["Wo"], np.float32)
    bo = np.asarray(inputs["bo"], np.float32)

    perm = plan["perm"]

    # ---- per-core dense inputs ----
    xr = np.concatenate([x[:, CODE_DIM:], CODE_WEIGHT * x[:, :CODE_DIM]], axis=1)
    xpad = np.zeros((NPAD, IN_DIM), np.float32)
    xpad[perm] = xr
    # [C, T, P(feat-within-chunk), KPROJ*P(node)] so one DMA loads a tile's
    # whole lhsT set
    xtt = (
        xpad.reshape(NCORES, T, P, KPROJ, P)
        .transpose(0, 1, 4, 3, 2)
        .reshape(NCORES, T, P, KPROJ * P)
        .astype(NPBF)
    )

    # ---- algebraic collapse of the network tail ----
    # C2 = [per-head W2*Wo | W2@Asrc2 | W2@Adst2]  (256 x 12)
    W2y = np.stack(
        [W2[:, h * CH : (h + 1) * CH] @ Wo[h * CH : (h + 1) * CH, 0] for h in range(HEADS)],
        axis=1,
    )
    C2 = np.concatenate([W2y, W2 @ _asrc_mat(att_src2), W2 @ _asrc_mat(att_dst2)], axis=1)
    # G carry: W1G[:, h*12+j] = W1[:, hC:(h+1)C] @ C2[hC:(h+1)C, j]
    W1G = np.zeros((HIDDEN, NG), np.float32)
    for h in range(HEADS):
        W1G[:, h * NC2 : (h + 1) * NC2] = (
            W1[:, h * CH : (h + 1) * CH] @ C2[h * CH : (h + 1) * CH, :]
        )
    d1aug = np.concatenate(
        [W1G, W1 @ _asrc_mat(att_src1), W1 @ _asrc_mat(att_dst1)], axis=1
    )
    b1c2 = b1 @ C2  # [12]
    sc_bias = float(b2 @ Wo[:, 0] + bo[0])

    # ---- per-node relation histogram: RW[n, r] = sum of w_e over in-edges ----
    RW = np.zeros((NPAD, NRELP), np.float32)
    np.add.at(RW, (perm[edge_index[1].astype(np.int64)], edge_type), edge_weight)

    key = (plan["K_rel"], plan["K_gat"], tuple(sorted((probe or {}).items())))
    if key not in _CACHE:
        _CACHE[key] = _build_bass(
            plan["K_rel"], plan["K_gat"], plan["offs_rel"], plan["offs_gat"], probe
        )
    nc = _CACHE[key]

    common = dict(
        wp=np.ascontiguousarray(Wp.reshape(KPROJ, P, HIDDEN)).astype(NPBF),
        bp_row=bp.reshape(1, HIDDEN),
        d1aug=np.ascontiguousarray(
            np.stack([d1aug[:P], d1aug[P:]], axis=0)
        ),
        b1c2_rep=np.ascontiguousarray(np.broadcast_to(b1c2, (P, NC2))),
        rel_emb=np.concatenate(
            [rel_emb, np.zeros((NRELP - NREL, HIDDEN), np.float32)]
        ),
        sc_bias=np.full((P, 1), sc_bias, np.float32),
        mrow=_make_mrow(),
    )
    in_maps = []
    for c in range(NCORES):
        in_maps.append(
            dict(
                common,
                xtt=xtt[c],
                rwT=np.ascontiguousarray(RW[c * NP : (c + 1) * NP, :].T),
                eidx_rel=plan["eidx_rel"][c],
                eidx_gat=plan["eidx_gat"][c],
            )
        )
    return in_maps, nc, perm


def kernel(x, edge_index, **rest):
    inputs = dict(rest, x=x, edge_index=edge_index)
    edge_index = np.asarray(edge_index, np.int32)
    plan = _build_plan(edge_index)
    in_maps, nc, perm = prepare(inputs, plan)

    import os

    trace = bool(os.environ.get("GAT_TRACE"))
    res = run_bass_kernel_spmd(
        nc, in_maps, core_ids=list(range(NCORES)), trace=trace
    )
    global _LAST_RESULT
    _LAST_RESULT = res
    scores_pad = np.concatenate([r["score"] for r in res.results])
    return scores_pad[perm].astype(np.float32)


_LAST_RESULT = None
